# revision 1
# baseline (speedup 1.0000x reference)
"""Trainium2 Bass kernel for nn_GSAttention (spatial-reduction attention).

Strategy
--------
* Queries (N=4096) are sharded 512-per-core across 8 NeuronCores; batch
  (B=2) is kept on-core (each core handles its 512-query slice of BOTH
  batches: 1024 query rows).
* Key insight: the custom causal mask visits only reduced tokens
  m < vis(n) with max(vis) = 94, so only the first 94 (padded to 96) of
  the 1024 reduced KV tokens are ever attended to.  The whole KV path
  (strided conv + LN + KV projection) is computed for 96 tokens instead
  of 1024 (replicated on every core - it is tiny).
* Everything on-chip runs in a feature-major ("transposed") layout so
  every matmul contraction lands on the partition dimension with zero
  on-device transposes.  The host pre-packs all operands in the layout
  the device wants (weight transposes, im2col of the 6-row input strip,
  RoPE cos/sin tables, additive mask, bias broadcast).
* RoPE is evaluated as rot(q) = C*q0 -/+ S*q1 on full-width feature
  blocks.  The head dim is permuted (pair-first elements in rows
  0..383, pair-second in rows 384..767) so the DVE ops run at full
  128-partition width; QK^T then accumulates the two 32-row halves into
  one PSUM tile.  The same permutation is applied to q and k, leaving
  q.k invariant.
* Softmax is un-normalized exp (value range makes max-subtraction
  unnecessary); the denominator is obtained by augmenting V with a
  ones-column (row 64 of the AV PSUM output), broadcast across
  partitions on GPSIMD, reciprocal on DVE, fused into the PSUM->SBUF
  copy of the attention output.
"""

import os
import sys

for _p in ("/opt/trn_rl_repo", "/root/.axon_site/_ro/trn_rl_repo"):
    if os.path.isdir(_p) and _p not in sys.path:
        sys.path.insert(0, _p)

from contextlib import ExitStack

import numpy as np

# This container lacks the axon NTFF profile hook; shim it so trace=True
# degrades to a plain (untraced) run instead of crashing.
import types as _types
if "antenv.axon_hooks" not in sys.modules:
    _axh = _types.ModuleType("antenv.axon_hooks")
    _axh.get_axon_ntff_profile_hook = lambda: None
    sys.modules["antenv.axon_hooks"] = _axh

import concourse.bacc as bacc
import concourse.mybir as mybir
from concourse.tile import TileContext
from concourse.bass_utils import run_bass_kernel_spmd

F32 = mybir.dt.float32
F32R = mybir.dt.float32r
AF = mybir.ActivationFunctionType
ALU = mybir.AluOpType

# Problem constants (hardcoded per contest rules).
N_CORES = 8
B = 2
N = 4096
C = 768
HEADS = 12
HD = 64
SR = 2
H = W = 64
NQ = 512            # queries per core per batch
NQT = B * NQ        # query rows per core
M = 96              # padded visible reduced tokens (real max vis = 94)
M2 = B * M
MP = 256            # conv im2col columns, padded for fp32r full rate
KC = C * SR * SR    # 3072 conv contraction
SCALE = 1.0 / 8.0
NEG = -1e30
NCH = C // 128      # 6 feature chunks

USE_F32R = True     # fp32r matmuls: 4x PE throughput at free-dim >= 256


OPDT = F32R if USE_F32R else F32


def _mm(ap):
    return ap


def build_program():
    nc = bacc.Bacc("TRN2", target_bir_lowering=False, debug=False,
                   num_devices=N_CORES)

    def par(name, shape, out=False, dt=F32):
        return nc.declare_dram_parameter(name, list(shape), dt, isOutput=out)

    xT = par("xT", (C, NQT), dt=OPDT)
    xi2c = par("xi2c", (KC, MP), dt=OPDT)
    wqT = par("wqT", (C, C), dt=OPDT)
    wkT = par("wkT", (C, C), dt=OPDT)
    wvT = par("wvT", (C, C), dt=OPDT)
    srwT = par("srwT", (KC, C), dt=OPDT)
    projT = par("projT", (C, C), dt=OPDT)
    cq = par("cq", (128, NQT))
    sq = par("sq", (128, NQT))
    ck = par("ck", (128, M2))
    sk = par("sk", (128, M2))
    maskS = par("maskS", (M, NQ), dt=OPDT)
    srb = par("srb", (C, 1))
    lng = par("lng", (C, 1))
    lnb = par("lnb", (C, 1))
    pbias = par("pbias", (128, C))
    Y = par("y", (NQT, C), out=True)

    with TileContext(nc) as tc, ExitStack() as st:
        pers = st.enter_context(tc.tile_pool(name="pers", bufs=1))

        # ---- persistent tiles -----------------------------------------
        cq_t = pers.tile([128, NQT], F32, tag="cq", name="cq")
        sq_t = pers.tile([128, NQT], F32, tag="sq", name="sq")
        ck_t = pers.tile([128, M2], F32, tag="ck", name="ck")
        sk_t = pers.tile([128, M2], F32, tag="sk", name="sk")
        mask_t = pers.tile([M, NQ], OPDT, tag="mask", name="mask")
        srb_t = [pers.tile([128, 1], F32, tag=f"srb{i}", name=f"srb{i}") for i in range(NCH)]
        lng_t = [pers.tile([128, 1], F32, tag=f"lng{i}", name=f"lng{i}") for i in range(NCH)]
        lnb_t = [pers.tile([128, 1], F32, tag=f"lnb{i}", name=f"lnb{i}") for i in range(NCH)]
        rotk = [pers.tile([128, M2], OPDT, tag=f"rotk{i}", name=f"rotk{i}") for i in range(NCH)]
        vaug = [pers.tile([M, HEADS * (HD + 1)], OPDT, tag=f"vaug{b}", name=f"vaug{b}")
                for b in range(B)]
        rotq = [pers.tile([128, NQT], OPDT, tag=f"rotq{i}", name=f"rotq{i}") for i in range(NCH)]
        attnT = [pers.tile([128, NQT], OPDT, tag=f"attnT{i}", name=f"attnT{i}")
                 for i in range(NCH)]
        ones_t = pers.tile([128, 1], F32, tag="ones", name="ones")

        nc.sync.dma_start(out=cq_t[:], in_=cq[:])
        nc.sync.dma_start(out=sq_t[:], in_=sq[:])
        nc.sync.dma_start(out=ck_t[:], in_=ck[:])
        nc.sync.dma_start(out=sk_t[:], in_=sk[:])
        nc.sync.dma_start(out=mask_t[:], in_=maskS[:])
        for i in range(NCH):
            r = slice(i * 128, (i + 1) * 128)
            nc.sync.dma_start(out=srb_t[i][:], in_=srb[r, :])
            nc.sync.dma_start(out=lng_t[i][:], in_=lng[r, :])
            nc.sync.dma_start(out=lnb_t[i][:], in_=lnb[r, :])
        nc.vector.memset(ones_t[:], 1.0)

        # ======= Phases C+A+B fused for overlap =======================
        # Emission order: q-projection matmuls first (PE starts as soon as
        # xT/wqT land), conv accumulation overlaps the srwT stream, then
        # LN -> K/V.  PSUM budget: 2 banks (q) + 6 banks (conv) = 8; the
        # LN-sum / K / V psum tiles recycle the conv banks via tag reuse.
        with tc.tile_pool(name="phC", bufs=1) as pC, \
             tc.tile_pool(name="phCt", bufs=1) as pCt, \
             tc.tile_pool(name="psC", bufs=1, space="PSUM") as psC, \
             tc.tile_pool(name="phA", bufs=3) as pA, \
             tc.tile_pool(name="phA1", bufs=1) as pA1, \
             tc.tile_pool(name="phBw", bufs=1) as pBw, \
             tc.tile_pool(name="psA", bufs=1, space="PSUM") as psA:
            # ---- DMAs for the q path first ----
            x_t = [pC.tile([128, NQT], OPDT, tag=f"x{i}", name=f"x{i}") for i in range(NCH)]
            wq_t = [pC.tile([128, C], OPDT, tag=f"wq{i}", name=f"wq{i}") for i in range(NCH)]
            for i in range(NCH):
                r = slice(i * 128, (i + 1) * 128)
                nc.sync.dma_start(out=x_t[i][:], in_=xT[r, :])
                nc.sync.dma_start(out=wq_t[i][:], in_=wqT[r, :])
            wk_t = [pBw.tile([128, C], OPDT, tag=f"wk{i}", name=f"wk{i}") for i in range(NCH)]
            wv_t = [pBw.tile([128, C], OPDT, tag=f"wv{i}", name=f"wv{i}") for i in range(NCH)]
            for i in range(NCH):
                r = slice(i * 128, (i + 1) * 128)
                nc.sync.dma_start(out=wk_t[i][:], in_=wkT[r, :])
                nc.sync.dma_start(out=wv_t[i][:], in_=wvT[r, :])

            # ---- interleaved: conv-chunk stream + q-projection blocks ----
            # One q block (12 matmuls, ~2.6us PE) per two conv chunks keeps
            # PE busy while srwT streams from HBM.
            xr_ps = [psA.tile([128, MP], F32, tag=f"xr{o}", name=f"xr{o}") for o in range(NCH)]

            def conv_chunk(kc):
                srw_l = pA.tile([128, C], OPDT, tag="srw", name="srw")
                xi_l = pA.tile([128, MP], OPDT, tag="xi", name="xi")
                kr = slice(kc * 128, (kc + 1) * 128)
                nc.sync.dma_start(out=srw_l[:], in_=srwT[kr, :])
                nc.sync.dma_start(out=xi_l[:], in_=xi2c[kr, :])
                for o in range(NCH):
                    nc.tensor.matmul(
                        xr_ps[o][:], _mm(srw_l[:, o * 128:(o + 1) * 128]),
                        _mm(xi_l[:]), start=(kc == 0), stop=(kc == KC // 128 - 1))

            def q_block(o, nh):
                ns = slice(nh * NQ, (nh + 1) * NQ)
                psA_ = psC.tile([128, NQ], F32, tag="qa", name="qa")
                psB_ = psC.tile([128, NQ], F32, tag="qb", name="qb")
                for cc in range(NCH):
                    nc.tensor.matmul(
                        psA_[:], _mm(wq_t[cc][:, o * 128:(o + 1) * 128]),
                        _mm(x_t[cc][:, ns]),
                        start=(cc == 0), stop=(cc == NCH - 1))
                for cc in range(NCH):
                    nc.tensor.matmul(
                        psB_[:],
                        _mm(wq_t[cc][:, (o + 3) * 128:(o + 4) * 128]),
                        _mm(x_t[cc][:, ns]),
                        start=(cc == 0), stop=(cc == NCH - 1))
                t1 = pCt.tile([128, NQ], F32, tag="qt1", name="qt1")
                t2 = pCt.tile([128, NQ], F32, tag="qt2", name="qt2")
                nc.vector.tensor_mul(t1[:], psA_[:], cq_t[:, ns])
                nc.vector.tensor_mul(t2[:], psB_[:], sq_t[:, ns])
                nc.vector.tensor_sub(rotq[o][:, ns], t1[:], t2[:])
                t3 = pCt.tile([128, NQ], F32, tag="qt3", name="qt3")
                t4 = pCt.tile([128, NQ], F32, tag="qt4", name="qt4")
                nc.vector.tensor_mul(t3[:], psB_[:], cq_t[:, ns])
                nc.vector.tensor_mul(t4[:], psA_[:], sq_t[:, ns])
                nc.vector.tensor_add(rotq[o + 3][:, ns], t3[:], t4[:])

            for step in range(6):
                for j in range(4):
                    conv_chunk(4 * step + j)
                q_block(step // 2, step % 2)

            xr_sb = [pA1.tile([128, M2], F32, tag=f"xrs{o}", name=f"xrs{o}") for o in range(NCH)]
            for o in range(NCH):
                nc.scalar.activation(xr_sb[o][:], xr_ps[o][:, :M2],
                                     AF.Identity, bias=srb_t[o][:])

            # LN statistics via ones-matmul partition reduction
            # (psum tiles recycle conv banks by tag reuse)
            sum_ps = psA.tile([1, M2], F32, tag="xr0", name="sum")
            ssq_ps = psA.tile([1, M2], F32, tag="xr1", name="ssq")
            for o in range(NCH):
                nc.tensor.matmul(sum_ps[:], ones_t[:], xr_sb[o][:],
                                 start=(o == 0), stop=(o == NCH - 1))
            for o in range(NCH):
                sqt = pA.tile([128, M2], F32, tag="sqt", name="sqt")
                nc.vector.tensor_mul(sqt[:], xr_sb[o][:], xr_sb[o][:])
                nc.tensor.matmul(ssq_ps[:], ones_t[:], sqt[:],
                                 start=(o == 0), stop=(o == NCH - 1))
            mu = pA1.tile([1, M2], F32, tag="mu", name="mu")
            mu2 = pA1.tile([1, M2], F32, tag="mu2", name="mu2")
            var = pA1.tile([1, M2], F32, tag="var", name="var")
            std = pA1.tile([1, M2], F32, tag="std", name="std")
            istd = pA1.tile([1, M2], F32, tag="istd", name="istd")
            nc.scalar.mul(mu[:], sum_ps[:], 1.0 / C)
            nc.vector.tensor_mul(mu2[:], mu[:], mu[:])
            nc.vector.scalar_tensor_tensor(var[:], ssq_ps[:], 1.0 / C, mu2[:],
                                           ALU.mult, ALU.subtract)
            eps_t = pA1.tile([1, 1], F32, tag="eps", name="eps")
            nc.vector.memset(eps_t[:], 1e-5)
            nc.scalar.activation(std[:], var[:], AF.Sqrt, bias=eps_t[:])
            nc.vector.reciprocal(istd[:], std[:])
            mu_b = pA1.tile([128, M2], F32, tag="mu_b", name="mu_b")
            istd_b = pA1.tile([128, M2], F32, tag="istd_b", name="istd_b")
            nc.gpsimd.partition_broadcast(mu_b[:], mu[:])
            nc.gpsimd.partition_broadcast(istd_b[:], istd[:])

            xln = [pers.tile([128, MP], OPDT, tag=f"xln{o}", name=f"xln{o}") for o in range(NCH)]
            for o in range(NCH):
                t = pA.tile([128, M2], F32, tag="lnt", name="lnt")
                nc.vector.tensor_sub(t[:], xr_sb[o][:], mu_b[:])
                nc.vector.tensor_mul(t[:], t[:], istd_b[:])
                nc.vector.tensor_scalar(xln[o][:, :M2], t[:], lng_t[o][:],
                                        lnb_t[o][:], ALU.mult, ALU.add)
                nc.vector.memset(xln[o][:, M2:MP].bitcast(F32), 0.0)

            # ---- K projection + RoPE (psum via tag reuse) ----
            k_ps = [psA.tile([128, MP], F32, tag=f"xr{o}", name=f"k{o}") for o in range(NCH)]
            for o in range(NCH):
                for cc in range(NCH):
                    nc.tensor.matmul(
                        k_ps[o][:], _mm(wk_t[cc][:, o * 128:(o + 1) * 128]),
                        _mm(xln[cc][:]), start=(cc == 0), stop=(cc == NCH - 1))
            for o in range(3):
                t1 = pA.tile([128, M2], F32, tag="kt1", name="kt1")
                t2 = pA.tile([128, M2], F32, tag="kt2", name="kt2")
                nc.vector.tensor_mul(t1[:], k_ps[o][:, :M2], ck_t[:])
                nc.vector.tensor_mul(t2[:], k_ps[o + 3][:, :M2], sk_t[:])
                nc.vector.tensor_sub(rotk[o][:], t1[:], t2[:])
                t3 = pA.tile([128, M2], F32, tag="kt3", name="kt3")
                t4 = pA.tile([128, M2], F32, tag="kt4", name="kt4")
                nc.vector.tensor_mul(t3[:], k_ps[o + 3][:, :M2], ck_t[:])
                nc.vector.tensor_mul(t4[:], k_ps[o][:, :M2], sk_t[:])
                nc.vector.tensor_add(rotk[o + 3][:], t3[:], t4[:])

            # ---- V projection into the 65-col augmented layout ----
            for b in range(B):
                for half in range(2):
                    v_ps = psA.tile([M, 384], F32, tag=f"xr{2 * b + half + 2}",
                                    name=f"v{b}{half}")
                    for cc in range(NCH):
                        nc.tensor.matmul(
                            v_ps[:], _mm(xln[cc][:, b * M:(b + 1) * M]),
                            _mm(wv_t[cc][:, half * 384:(half + 1) * 384]),
                            start=(cc == 0), stop=(cc == NCH - 1))
                    dst = vaug[b][:].rearrange("p (h d) -> p h d", d=HD + 1)
                    src_ = v_ps[:].rearrange("p (h d) -> p h d", d=HD)
                    nc.vector.tensor_copy(
                        dst[:, half * 6:(half + 1) * 6, 0:HD], src_)
                ocol = vaug[b][:].rearrange("p (h d) -> p h d", d=HD + 1)
                nc.vector.memset(ocol[:, :, HD:HD + 1].bitcast(F32), 1.0)

        # ================= Phase D: attention =========================
        with tc.tile_pool(name="phD", bufs=3) as pD, \
             tc.tile_pool(name="psD", bufs=3, space="PSUM") as psD:
            for h in range(HEADS):
                hq, hr = h // 4, (h % 4) * 32
                rs = slice(hr, hr + 32)
                tp = (hr, 0) if hr == 96 else None
                for b in range(B):
                    ms = slice(b * M, (b + 1) * M)
                    qs = slice(b * NQ, (b + 1) * NQ)
                    z_ps = psD.tile([M, NQ], F32, tag="z", name="z")
                    nc.tensor.matmul(z_ps[:], _mm(rotk[hq][rs, ms]),
                                     _mm(rotq[hq][rs, qs]),
                                     start=True, stop=False, tile_position=tp)
                    nc.tensor.matmul(z_ps[:], _mm(rotk[hq + 3][rs, ms]),
                                     _mm(rotq[hq + 3][rs, qs]),
                                     start=False, stop=True, tile_position=tp)
                    e_sb = pD.tile([M, NQ], OPDT, tag="e", name="e")
                    nc.scalar.activation(e_sb[:], z_ps[:], AF.Exp, scale=SCALE)
                    nc.gpsimd.tensor_mul(e_sb[:], e_sb[:], mask_t[:])
                    u_ps = psD.tile([HD + 1, NQ], F32, tag="u", name="u")
                    vslice = vaug[b][:, h * (HD + 1):(h + 1) * (HD + 1)]
                    nc.tensor.matmul(u_ps[:], _mm(vslice), _mm(e_sb[:]),
                                     start=True, stop=True)
                    den = pD.tile([1, NQ], F32, tag="den", name="den")
                    nc.scalar.mul(den[:], u_ps[HD:HD + 1, :], 1.0)
                    den_b = pD.tile([HD, NQ], F32, tag="den_b", name="den_b")
                    nc.gpsimd.partition_broadcast(den_b[:], den[:])
                    rec = pD.tile([HD, NQ], F32, tag="rec", name="rec")
                    nc.vector.reciprocal_approx_fast(rec[:], den_b[:])
                    dst = attnT[h // 2][(h % 2) * 64:(h % 2) * 64 + 64, qs]
                    nc.vector.tensor_mul(dst, u_ps[0:HD, :], rec[:])

        # ================= Phase E: output projection =================
        with tc.tile_pool(name="phE", bufs=1) as pE, \
             tc.tile_pool(name="phEy", bufs=3) as pEy, \
             tc.tile_pool(name="psE", bufs=4, space="PSUM") as psE:
            pj_t = [pE.tile([128, C], OPDT, tag=f"pj{i}", name=f"pj{i}") for i in range(NCH)]
            pb_t = pE.tile([128, C], F32, tag="pb", name="pb")
            for i in range(NCH):
                r = slice(i * 128, (i + 1) * 128)
                nc.sync.dma_start(out=pj_t[i][:], in_=projT[r, :])
            nc.sync.dma_start(out=pb_t[:], in_=pbias[:])
            for nk in range(NQT // 128):
                ncs = slice(nk * 128, (nk + 1) * 128)
                y_sb = pEy.tile([128, C], F32, tag="y", name="y")
                for ph in range(2):
                    pcs = slice(ph * 384, (ph + 1) * 384)
                    y_ps = psE.tile([128, 384], F32, tag="yp", name="yp")
                    for oc in range(NCH):
                        nc.tensor.matmul(y_ps[:], _mm(attnT[oc][:, ncs]),
                                         _mm(pj_t[oc][:, pcs]),
                                         start=(oc == 0), stop=(oc == NCH - 1))
                    nc.vector.tensor_add(y_sb[:, pcs], y_ps[:], pb_t[:, pcs])
                nc.sync.dma_start(out=Y[ncs, :], in_=y_sb[:])

    nc.compile()
    return nc


# ======================= host-side preparation =======================

def _angles(dim, end, w, step=1.0, bias=0.0, theta=10000.0):
    flat = np.arange(end, dtype=np.float32)
    xp = (bias + (flat % w) * step).astype(np.float32)
    yp = (bias + (flat // w) * step).astype(np.float32)
    freqs = (1.0 / theta ** (np.arange(0, dim, 4, dtype=np.float32)[: dim // 4]
                             / dim)).astype(np.float32)
    xf = np.outer(xp, freqs)
    yf = np.outer(yp, freqs)
    return np.stack([xf, yf], axis=-1).reshape(end, -1).astype(np.float32)


def _host_prep(x, Wq, Wkv, sr_w, sr_b, ln_g, ln_b, proj_w, proj_b):
    """Build per-core input maps (all float32, device-ready layouts)."""
    f = np.float32
    x = np.asarray(x, f)
    Wq = np.asarray(Wq, f)
    Wkv = np.asarray(Wkv, f)
    sr_w = np.asarray(sr_w, f)
    proj_w = np.asarray(proj_w, f)

    # head-dim permutation: pair-first -> rows 0..383, pair-second -> 384..767
    hh = np.arange(HEADS)[:, None] * HD
    jj = np.arange(HD // 2)[None, :] * 2
    perm = np.concatenate([(hh + jj).ravel(), (hh + jj + 1).ravel()])

    wqT = np.ascontiguousarray(Wq[perm, :].T)
    wkT = np.ascontiguousarray(Wkv[:C][perm, :].T)
    wvT = np.ascontiguousarray(Wkv[C:].T)
    srwT = np.ascontiguousarray(sr_w.reshape(C, KC).T)
    projT = np.ascontiguousarray(proj_w.T)

    # im2col of the first 6 image rows, both batches: [3072, 256]
    strip = x[:, :6 * W, :].reshape(B, 3, 2, 32, 2, C)   # b, i, di, j, dj, c
    xi2c = np.zeros((KC, MP), f)
    xi2c[:, :M2] = strip.transpose(5, 2, 4, 0, 1, 3).reshape(KC, M2)

    # RoPE tables
    ang_q = _angles(HD, N, W)                            # [4096, 32]
    ang_k = _angles(HD, N // (SR * SR), W, step=SR, bias=1.0 - 1.0 / SR)
    rowj = np.arange(128) % 32
    cq_full = np.cos(ang_q)[:, rowj].T                   # [128, 4096]
    sq_full = np.sin(ang_q)[:, rowj].T
    ckk = np.cos(ang_k)[:M, rowj].T                      # [128, 96]
    skk = np.sin(ang_k)[:M, rowj].T
    ck2 = np.ascontiguousarray(np.concatenate([ckk, ckk], 1))
    sk2 = np.ascontiguousarray(np.concatenate([skk, skk], 1))

    # visibility mask (additive, pre-scale units: exp(SCALE*(z+mask)))
    n_all = np.arange(N)
    xpos = n_all // (SR * H)
    ox = n_all // H
    oy = n_all % H
    ypos = (ox + oy * H) // (SR * H)
    vis = xpos * SR + ypos + 1                            # [4096]

    pbias = np.ascontiguousarray(
        np.broadcast_to(np.asarray(proj_b, f)[None, :], (128, C)))
    srb = np.ascontiguousarray(np.asarray(sr_b, f)[:, None])
    lng = np.ascontiguousarray(np.asarray(ln_g, f)[:, None])
    lnb = np.ascontiguousarray(np.asarray(ln_b, f)[:, None])

    shared = dict(xi2c=xi2c, wqT=wqT, wkT=wkT, wvT=wvT, srwT=srwT,
                  projT=projT, ck=ck2, sk=sk2, srb=srb, lng=lng, lnb=lnb,
                  pbias=pbias)

    in_maps = []
    for core in range(N_CORES):
        ns = slice(core * NQ, (core + 1) * NQ)
        xs = x[:, ns, :]                                  # [2, 512, 768]
        xT = np.ascontiguousarray(xs.transpose(2, 0, 1).reshape(C, NQT))
        cqc = np.ascontiguousarray(
            np.concatenate([cq_full[:, ns]] * B, axis=1))
        sqc = np.ascontiguousarray(
            np.concatenate([sq_full[:, ns]] * B, axis=1))
        mask = (np.arange(M)[:, None] < vis[ns][None, :]).astype(f)
        in_maps.append(dict(shared, xT=xT, cq=cqc, sq=sqc, maskS=mask))
    return in_maps


_NC_CACHE = {}


def _get_program():
    if "nc" not in _NC_CACHE:
        _NC_CACHE["nc"] = build_program()
    return _NC_CACHE["nc"]


def kernel(x, Wq, Wkv, sr_w, sr_b, ln_g, ln_b, proj_w, proj_b, H=None, W=None,
           _trace=False):
    nc = _get_program()
    in_maps = _host_prep(x, Wq, Wkv, sr_w, sr_b, ln_g, ln_b, proj_w, proj_b)
    res = run_bass_kernel_spmd(nc, in_maps, list(range(N_CORES)),
                               trace=_trace)
    kernel.last_result = res
    out = np.empty((B, N, C), np.float32)
    for core in range(N_CORES):
        y = res.results[core]["y"].reshape(B, NQ, C)
        out[:, core * NQ:(core + 1) * NQ, :] = y
    return out



# revision 13
# speedup vs baseline: 1.4300x; 1.4300x over previous
"""Trainium2 Bass kernel for nn_GSAttention (spatial-reduction attention).

Strategy (v1, bf16)
-------------------
* Queries sharded 512/core over 8 cores; each core handles both batches
  (1024 query rows).  KV path (conv+LN+KV proj) replicated per core but
  only for the 96 reduced tokens the causal mask can ever see.
* All matmul operands are bf16 (1 cyc/row on PE, half the HBM traffic of
  fp32); accumulation stays fp32 in PSUM.  Host pre-packs every operand
  in device layout; all small tables ride in one fused DMA blob.
* LN affine (g, b) is folded into the K/V projection weights on the host;
  the V-side bias collapses into the final projection bias, which is
  added on the host after the gather (it is exact: attention rows sum
  to 1).  K-side bias is applied on-device per-partition during the
  PSUM->SBUF copy.
* The spatial-causal mask is applied as a third matmul accumulated into
  the QK PSUM tile: lhsT = triangular NEG matrix A [96,96], rhs = onehot
  B[j,n] = [vis(n)==j], so z += A[vis(n),m] = NEG*[m>=vis(n)].
* Softmax normalization: V is augmented with a ones column (row 64 of
  the AV output = denominator); 1/den via DVE reciprocal [1,512] -> PE
  ones-matmul broadcast to [64,512] PSUM -> one tensor-mul (DVE/Pool
  alternating) writes the normalized, bf16 attention output.
"""

import os
import sys

for _p in ("/opt/trn_rl_repo", "/root/.axon_site/_ro/trn_rl_repo"):
    if os.path.isdir(_p) and _p not in sys.path:
        sys.path.insert(0, _p)

from contextlib import ExitStack

import numpy as np
import ml_dtypes

import types as _types
if "antenv.axon_hooks" not in sys.modules:
    _axh = _types.ModuleType("antenv.axon_hooks")
    _axh.get_axon_ntff_profile_hook = lambda: None
    sys.modules["antenv.axon_hooks"] = _axh

import concourse.bacc as bacc
import concourse.mybir as mybir
from concourse.tile import TileContext
from concourse.bass_utils import run_bass_kernel_spmd

F32 = mybir.dt.float32
BF16 = mybir.dt.bfloat16
AF = mybir.ActivationFunctionType
ALU = mybir.AluOpType
BF = ml_dtypes.bfloat16

# Problem constants (hardcoded).
N_CORES = 8
B = 2
N = 4096
C = 768
HEADS = 12
HD = 64
SR = 2
H = W = 64
NQ = 512            # queries per core per batch
NQT = B * NQ        # query rows per core
M = 96              # padded visible reduced tokens (real max vis = 94)
M2 = B * M
KC = C * SR * SR    # 3072 conv contraction
SCALE = 1.0 / 8.0
NEG = -60000.0
NCH = C // 128      # 6 feature chunks

# blob16 column offsets
O_CQ = 0
O_SQ = O_CQ + NQT
O_CK = O_SQ + NQT
O_SK = O_CK + M2
O_A = O_SK + M2
O_B = O_A + M
BLOB16 = O_B + NQ


def build_program():
    nc = bacc.Bacc("TRN2", target_bir_lowering=False, debug=False,
                   num_devices=N_CORES)

    def par(name, shape, out=False, dt=F32):
        return nc.declare_dram_parameter(name, list(shape), dt, isOutput=out)

    xT = par("xT", (C, NQT), dt=BF16)
    xi2c = par("xi2c", (KC, 256), dt=BF16)
    wqT = par("wqT", (C, C), dt=BF16)
    wkT = par("wkT", (C, C), dt=BF16)
    wvT = par("wvT", (C, C), dt=BF16)
    srwT = par("srwT", (KC, C), dt=BF16)
    projT = par("projT", (C, C), dt=BF16)
    blob16 = par("blob16", (128, BLOB16), dt=BF16)
    blob32 = par("blob32", (128, 12), dt=F32)
    Y = par("y", (NQT, C), out=True, dt=BF16)

    with TileContext(nc) as tc, ExitStack() as st:
        st.enter_context(nc.allow_low_precision(
            reason="bf16 reciprocal of softmax denominator; rel tol 2e-2"))
        pers = st.enter_context(tc.tile_pool(name="pers", bufs=1))

        # ---- persistent tiles -----------------------------------------
        b16 = pers.tile([128, BLOB16], BF16, tag="b16", name="b16")
        b32 = pers.tile([128, 12], F32, tag="b32", name="b32")
        rotq = [pers.tile([128, NQT], BF16, tag=f"rotq{i}", name=f"rotq{i}")
                for i in range(NCH)]
        rotk = [pers.tile([128, M2], BF16, tag=f"rotk{i}", name=f"rotk{i}")
                for i in range(NCH)]
        vaug = [pers.tile([M, HEADS * (HD + 1)], BF16, tag=f"vaug{b}",
                          name=f"vaug{b}") for b in range(B)]
        attnT = [pers.tile([128, NQT], BF16, tag=f"attnT{i}", name=f"attnT{i}")
                 for i in range(NCH)]
        xln = [pers.tile([128, M2], BF16, tag=f"xln{o}", name=f"xln{o}")
               for o in range(NCH)]
        ones_t = pers.tile([128, 1], BF16, tag="ones", name="ones")
        one1_t = pers.tile([1, 128], BF16, tag="one1", name="one1")
        eps_t = pers.tile([1, 1], F32, tag="eps", name="eps")

        nc.vector.memset(ones_t[:], 1.0)
        nc.vector.memset(one1_t[:], 1.0)
        nc.vector.memset(eps_t[:], 1e-5)

        cq_s = b16[:, O_CQ:O_CQ + NQT]
        sq_s = b16[:, O_SQ:O_SQ + NQT]
        ck_s = b16[:, O_CK:O_CK + M2]
        sk_s = b16[:, O_SK:O_SK + M2]
        A_s = b16[0:M, O_A:O_A + M]
        B_s = b16[0:M, O_B:O_B + NQ]

        with tc.tile_pool(name="pIn", bufs=1) as pIn, \
             tc.tile_pool(name="pSrw", bufs=4) as pSrw, \
             tc.tile_pool(name="pQs", bufs=1) as pQs, \
             tc.tile_pool(name="pT", bufs=1) as pT, \
             tc.tile_pool(name="pS", bufs=2) as pS, \
             tc.tile_pool(name="psQ", bufs=2, space="PSUM") as psQ, \
             tc.tile_pool(name="psA", bufs=1, space="PSUM") as psA:

            # ---- fused input DMAs (SP queue, bandwidth-ordered) -------
            def big_dma(tile, dram_ap, a, k):
                nc.sync.dma_start(
                    out=tile[:].rearrange("p (a k) -> p a k", a=a),
                    in_=dram_ap.rearrange("(a p) k -> p a k", p=128))

            nc.sync.dma_start(out=b16[:], in_=blob16[:])
            nc.sync.dma_start(out=b32[:], in_=blob32[:])
            wq_t = pIn.tile([128, NCH * C], BF16, tag="wq", name="wq")
            big_dma(wq_t, wqT[:], NCH, C)
            xT_t = pIn.tile([128, NCH * NQT], BF16, tag="xT", name="xT")
            big_dma(xT_t, xT[:], NCH, NQT)
            xi_t = pIn.tile([128, 24 * 256], BF16, tag="xi", name="xi")
            big_dma(xi_t, xi2c[:], 24, 256)
            srw_t = []
            for s in range(4):
                t = pSrw.tile([128, NCH * C], BF16, tag="srw", name=f"srw{s}")
                big_dma(t, srwT[s * 768:(s + 1) * 768, :], NCH, C)
                srw_t.append(t)
            wk_t = pIn.tile([128, NCH * C], BF16, tag="wk", name="wk")
            big_dma(wk_t, wkT[:], NCH, C)
            wv_t = pIn.tile([128, NCH * C], BF16, tag="wv", name="wv")
            big_dma(wv_t, wvT[:], NCH, C)
            pj_t = pIn.tile([128, NCH * C], BF16, tag="pj", name="pj")
            big_dma(pj_t, projT[:], NCH, C)

            # ---- Q projection + RoPE ----------------------------------
            q_sb = [pQs.tile([128, NQT], BF16, tag=f"qsb{o}", name=f"qsb{o}")
                    for o in range(NCH)]

            def q_block(o, nh):
                ns = slice(nh * NQ, (nh + 1) * NQ)
                q_ps = psQ.tile([128, NQ], F32, tag="q", name=f"q{o}{nh}")
                for cc in range(NCH):
                    nc.tensor.matmul(
                        q_ps[:], wq_t[:, cc * C + o * 128: cc * C + (o + 1) * 128],
                        xT_t[:, cc * NQT + nh * NQ: cc * NQT + (nh + 1) * NQ],
                        start=(cc == 0), stop=(cc == NCH - 1))
                nc.scalar.copy(q_sb[o][:, ns], q_ps[:])

            def rope_pair(c):
                t1 = pT.tile([128, NQT], BF16, tag="t1", name="t1")
                t2 = pT.tile([128, NQT], BF16, tag="t2", name="t2")
                nc.vector.tensor_mul(t1[:], q_sb[c][:], cq_s)
                nc.vector.tensor_mul(t2[:], q_sb[c + 3][:], sq_s)
                nc.vector.tensor_sub(rotq[c][:], t1[:], t2[:])
                t3 = pT.tile([128, NQT], BF16, tag="t3", name="t3")
                t4 = pT.tile([128, NQT], BF16, tag="t4", name="t4")
                nc.vector.tensor_mul(t3[:], q_sb[c + 3][:], cq_s)
                nc.vector.tensor_mul(t4[:], q_sb[c][:], sq_s)
                nc.vector.tensor_add(rotq[c + 3][:], t3[:], t4[:])

            for c in range(3):
                for o in (c, c + 3):
                    q_block(o, 0)
                    q_block(o, 1)
                rope_pair(c)

            # ---- conv (spatial reduction) -----------------------------
            xr_ps = [psA.tile([128, M2], F32, tag=f"xr{o}", name=f"xr{o}")
                     for o in range(NCH)]
            for kc in range(24):
                s, w = kc // 6, kc % 6
                xi_sl = xi_t[:, kc * 256: kc * 256 + M2]
                for o in range(NCH):
                    nc.tensor.matmul(
                        xr_ps[o][:],
                        srw_t[s][:, w * C + o * 128: w * C + (o + 1) * 128],
                        xi_sl, start=(kc == 0), stop=(kc == 23))

            # ---- LN (stats via ones-matmul; affine folded on host) ----
            xr_sb = [pS.tile([128, M2], BF16, tag=f"xrs{o}", name=f"xrs{o}")
                     for o in range(NCH)]
            for o in range(NCH):
                nc.scalar.activation(xr_sb[o][:], xr_ps[o][:], AF.Identity,
                                     bias=b32[:, o:o + 1])
            sum_ps = psA.tile([1, M2], F32, tag="xr0", name="sum")
            for o in range(NCH):
                nc.tensor.matmul(sum_ps[:], ones_t[:], xr_sb[o][:],
                                 start=(o == 0), stop=(o == NCH - 1))
            ssq_ps = psA.tile([1, M2], F32, tag="xr1", name="ssq")
            for o in range(NCH):
                sqt = pS.tile([128, M2], BF16, tag="sqt", name="sqt")
                nc.vector.tensor_mul(sqt[:], xr_sb[o][:], xr_sb[o][:])
                nc.tensor.matmul(ssq_ps[:], ones_t[:], sqt[:],
                                 start=(o == 0), stop=(o == NCH - 1))
            mu = pS.tile([1, M2], F32, tag="mu", name="mu")
            mu2 = pS.tile([1, M2], F32, tag="mu2", name="mu2")
            var = pS.tile([1, M2], F32, tag="var", name="var")
            std = pS.tile([1, M2], F32, tag="std", name="std")
            bsrc = pS.tile([1, 2 * M2], BF16, tag="bsrc", name="bsrc")
            nc.scalar.mul(mu[:], sum_ps[:], 1.0 / C)
            nc.vector.tensor_mul(mu2[:], mu[:], mu[:])
            nc.vector.scalar_tensor_tensor(var[:], ssq_ps[:], 1.0 / C, mu2[:],
                                           ALU.mult, ALU.subtract)
            nc.scalar.activation(std[:], var[:], AF.Sqrt, bias=eps_t[:])
            nc.vector.reciprocal(bsrc[:, 0:M2], std[:])
            nc.vector.scalar_tensor_tensor(bsrc[:, M2:2 * M2], mu[:], -1.0,
                                           bsrc[:, 0:M2], ALU.mult, ALU.mult)
            bc_ps = psA.tile([128, 2 * M2], F32, tag="xr2", name="bc")
            nc.tensor.matmul(bc_ps[:], one1_t[:], bsrc[:], start=True,
                             stop=True)
            bc_sb = pS.tile([128, 2 * M2], BF16, tag="bcs", name="bcs")
            nc.scalar.copy(bc_sb[:], bc_ps[:])
            for o in range(NCH):
                t = pS.tile([128, M2], BF16, tag="lnt", name="lnt")
                nc.vector.tensor_mul(t[:], xr_sb[o][:], bc_sb[:, 0:M2])
                nc.vector.tensor_add(xln[o][:], t[:], bc_sb[:, M2:2 * M2])

            # ---- K projection (+bias) + RoPE --------------------------
            k_ps = [psA.tile([128, M2], F32, tag=f"xr{o}", name=f"k{o}")
                    for o in range(NCH)]
            for o in range(NCH):
                for cc in range(NCH):
                    nc.tensor.matmul(
                        k_ps[o][:], wk_t[:, cc * C + o * 128: cc * C + (o + 1) * 128],
                        xln[cc][:], start=(cc == 0), stop=(cc == NCH - 1))
            k_sb = [pS.tile([128, M2], BF16, tag=f"ksb{o}", name=f"ksb{o}")
                    for o in range(NCH)]
            for o in range(NCH):
                nc.scalar.activation(k_sb[o][:], k_ps[o][:], AF.Identity,
                                     bias=b32[:, 6 + o:7 + o])
            for c in range(3):
                t1 = pS.tile([128, M2], BF16, tag="kt1", name="kt1")
                t2 = pS.tile([128, M2], BF16, tag="kt2", name="kt2")
                nc.vector.tensor_mul(t1[:], k_sb[c][:], ck_s)
                nc.vector.tensor_mul(t2[:], k_sb[c + 3][:], sk_s)
                nc.vector.tensor_sub(rotk[c][:], t1[:], t2[:])
                t3 = pS.tile([128, M2], BF16, tag="kt3", name="kt3")
                t4 = pS.tile([128, M2], BF16, tag="kt4", name="kt4")
                nc.vector.tensor_mul(t3[:], k_sb[c + 3][:], ck_s)
                nc.vector.tensor_mul(t4[:], k_sb[c][:], sk_s)
                nc.vector.tensor_add(rotk[c + 3][:], t3[:], t4[:])

            # ---- V projection into 65-col augmented layout ------------
            for b in range(B):
                for half in range(2):
                    v_ps = psA.tile([M, 384], F32, tag=f"xr{2 * b + half}",
                                    name=f"v{b}{half}")
                    for cc in range(NCH):
                        nc.tensor.matmul(
                            v_ps[:], xln[cc][:, b * M:(b + 1) * M],
                            wv_t[:, cc * C + half * 384: cc * C + (half + 1) * 384],
                            start=(cc == 0), stop=(cc == NCH - 1))
                    dst = vaug[b][:].rearrange("p (h d) -> p h d", d=HD + 1)
                    src_ = v_ps[:].rearrange("p (h d) -> p h d", d=HD)
                    nc.vector.tensor_copy(
                        dst[:, half * 6:(half + 1) * 6, 0:HD], src_)
                ocol = vaug[b][:].rearrange("p (h d) -> p h d", d=HD + 1)
                nc.vector.memset(ocol[:, :, HD:HD + 1], 1.0)

        # ================= Phase D: attention =========================
        # Per (batch, head): z = k^T q + mask (3 accumulated matmuls),
        # e = exp(z/8) [Act], u = V_aug^T e [PE] (row 64 = denominator),
        # rec = 1/den [DVE, bf16 SBUF].  Pairs of rec rows bounce through a
        # DRAM tile and return partition-broadcast to [64, 2*NQ] SBUF (DMA
        # engines are idle in this phase); one tensor-mul per unit
        # (DVE/Pool alternating) then writes normalized bf16 attnT.
        units = [(b, h) for b in range(B) for h in range(HEADS)]
        NP = len(units) // 2
        with tc.tile_pool(name="pD", bufs=3) as pD, \
             tc.tile_pool(name="pDr", bufs=4, space="DRAM") as pDr, \
             tc.tile_pool(name="pBc", bufs=3) as pBc, \
             tc.tile_pool(name="psD", bufs=1, space="PSUM") as psD:
            us, bcs = {}, {}
            for p in range(NP + 2):
                if p < NP:
                    rp = pD.tile([64, NQ], BF16, tag="rp", name="rp")
                    for k in range(2):
                        i = 2 * p + k
                        b, h = units[i]
                        hq, hr = h // 4, (h % 4) * 32
                        rs = slice(hr, hr + 32)
                        ms = slice(b * M, (b + 1) * M)
                        qs = slice(b * NQ, (b + 1) * NQ)
                        z_ps = psD.tile([M, NQ], F32, tag=f"z{i % 2}",
                                        name="z")
                        tp = (hr, 0) if hr == 96 else None
                        nc.tensor.matmul(z_ps[:], rotk[hq][rs, ms],
                                         rotq[hq][rs, qs], start=True,
                                         stop=False, tile_position=tp)
                        nc.tensor.matmul(z_ps[:], rotk[hq + 3][rs, ms],
                                         rotq[hq + 3][rs, qs], start=False,
                                         stop=False, tile_position=tp)
                        nc.tensor.matmul(z_ps[:], A_s, B_s, start=False,
                                         stop=True)
                        e_sb = pD.tile([M, NQ], BF16, tag="e", name="e")
                        nc.scalar.activation(e_sb[:], z_ps[:], AF.Exp,
                                             scale=SCALE)
                        u_ps = psD.tile([HD + 1, NQ], F32, tag=f"u{i % 6}",
                                        name="u")
                        nc.tensor.matmul(
                            u_ps[:],
                            vaug[b][:, h * (HD + 1):(h + 1) * (HD + 1)],
                            e_sb[:], start=True, stop=True)
                        nc.vector.reciprocal(rp[32 * k:32 * k + 1, :],
                                             u_ps[HD:HD + 1, :])
                        us[i] = u_ps
                    dr = pDr.tile([2, NQ], BF16, tag="dr", name="dr")
                    nc.sync.dma_start(
                        out=dr[:],
                        in_=rp[0:64, :].rearrange("(g r) n -> g r n",
                                                  r=32)[:, 0:1, :])
                    bc = pBc.tile([HD, 2 * NQ], BF16, tag="bc", name="bc")
                    nc.sync.dma_start(
                        out=bc[:].rearrange("p (g n) -> p g n", g=2),
                        in_=dr[:].rearrange("(o a) n -> o a n",
                                            o=1).broadcast_to((HD, 2, NQ)))
                    bcs[p] = bc
                if p >= 2:
                    q = p - 2
                    bc = bcs.pop(q)
                    for k in range(2):
                        j = 2 * q + k
                        b, h = units[j]
                        dst = attnT[h // 2][(h % 2) * HD:(h % 2 + 1) * HD,
                                            b * NQ:(b + 1) * NQ]
                        bsl = bc[:, k * NQ:(k + 1) * NQ]
                        nc.vector.tensor_mul(dst, us[j][0:HD, :], bsl)
                        del us[j]

        # ================= Phase E: output projection =================
        with tc.tile_pool(name="pE", bufs=3) as pE, \
             tc.tile_pool(name="psE", bufs=2, space="PSUM") as psE:
            for b in range(B):
                for nk in range(4):
                    r0 = b * NQ + nk * 128
                    y_sb = pE.tile([128, C], BF16, tag="y", name="y")
                    for ph in range(2):
                        y_ps = psE.tile([128, 384], F32, tag="yp", name="yp")
                        for oc in range(NCH):
                            nc.tensor.matmul(
                                y_ps[:], attnT[oc][:, r0:r0 + 128],
                                pj_t[:, oc * C + ph * 384: oc * C + (ph + 1) * 384],
                                start=(oc == 0), stop=(oc == NCH - 1))
                        nc.scalar.copy(y_sb[:, ph * 384:(ph + 1) * 384],
                                       y_ps[:])
                    nc.sync.dma_start(out=Y[r0:r0 + 128, :], in_=y_sb[:])

    nc.compile()
    return nc


# ======================= host-side preparation =======================

def _angles(dim, end, w, step=1.0, bias=0.0, theta=10000.0):
    flat = np.arange(end, dtype=np.float32)
    xp = (bias + (flat % w) * step).astype(np.float32)
    yp = (bias + (flat // w) * step).astype(np.float32)
    freqs = (1.0 / theta ** (np.arange(0, dim, 4, dtype=np.float32)[: dim // 4]
                             / dim)).astype(np.float32)
    xf = np.outer(xp, freqs)
    yf = np.outer(yp, freqs)
    return np.stack([xf, yf], axis=-1).reshape(end, -1).astype(np.float32)


def _host_prep(x, Wq, Wkv, sr_w, sr_b, ln_g, ln_b, proj_w, proj_b):
    f = np.float32
    x = np.asarray(x, f)
    Wq = np.asarray(Wq, f)
    Wkv = np.asarray(Wkv, f)
    sr_w = np.asarray(sr_w, f)
    proj_w = np.asarray(proj_w, f)
    sr_b = np.asarray(sr_b, f)
    ln_g = np.asarray(ln_g, f)
    ln_b = np.asarray(ln_b, f)
    proj_b = np.asarray(proj_b, f)

    # head-dim permutation: pair-first -> rows 0..383, pair-second -> 384..767
    hh = np.arange(HEADS)[:, None] * HD
    jj = np.arange(HD // 2)[None, :] * 2
    perm = np.concatenate([(hh + jj).ravel(), (hh + jj + 1).ravel()])

    Wk = Wkv[:C]
    Wv = Wkv[C:]
    wqT = np.ascontiguousarray(Wq[perm, :].T.astype(BF))
    wkT = np.ascontiguousarray((Wk * ln_g[None, :])[perm, :].T.astype(BF))
    wvT = np.ascontiguousarray((Wv * ln_g[None, :]).T.astype(BF))
    kbias = (Wk @ ln_b)[perm].astype(f)
    vbias = (Wv @ ln_b).astype(f)
    pbias_host = (proj_b + proj_w @ vbias).astype(f)
    srwT = np.ascontiguousarray(sr_w.reshape(C, KC).T.astype(BF))
    projT = np.ascontiguousarray(proj_w.T.astype(BF))

    # im2col of the first 6 image rows, both batches: [3072, 192] pad 256
    strip = x[:, :6 * W, :].reshape(B, 3, 2, 32, 2, C)   # b, i, di, j, dj, c
    xi2c = np.zeros((KC, 256), f)
    xi2c[:, :M2] = strip.transpose(5, 2, 4, 0, 1, 3).reshape(KC, M2)
    xi2c = xi2c.astype(BF)

    # RoPE tables
    ang_q = _angles(HD, N, W)
    ang_k = _angles(HD, N // (SR * SR), W, step=SR, bias=1.0 - 1.0 / SR)
    rowj = np.arange(128) % 32
    cq_full = np.cos(ang_q)[:, rowj].T                   # [128, 4096]
    sq_full = np.sin(ang_q)[:, rowj].T
    ckk = np.cos(ang_k)[:M, rowj].T                      # [128, 96]
    skk = np.sin(ang_k)[:M, rowj].T
    ck2 = np.concatenate([ckk, ckk], 1)
    sk2 = np.concatenate([skk, skk], 1)

    # visibility
    n_all = np.arange(N)
    xpos = n_all // (SR * H)
    ox = n_all // H
    oy = n_all % H
    ypos = (ox + oy * H) // (SR * H)
    vis = xpos * SR + ypos + 1                            # [4096], 1..94

    # mask matrices: A[j, m] = NEG*[m >= j]; B[j, n] = [vis(n) == j]
    A_m = np.zeros((128, M), f)
    jm = np.arange(M)
    A_m[:M, :] = np.where(jm[None, :] >= jm[:, None], NEG, 0.0)

    blob32 = np.zeros((128, 12), f)
    blob32[:, 0:6] = sr_b.reshape(NCH, 128).T
    blob32[:, 6:12] = kbias.reshape(NCH, 128).T

    shared = dict(xi2c=xi2c, wqT=wqT, wkT=wkT, wvT=wvT, srwT=srwT,
                  projT=projT, blob32=blob32)

    in_maps = []
    for core in range(N_CORES):
        ns = slice(core * NQ, (core + 1) * NQ)
        xs = x[:, ns, :]
        xTc = np.ascontiguousarray(
            xs.transpose(2, 0, 1).reshape(C, NQT).astype(BF))
        visc = vis[ns]
        B_m = np.zeros((128, NQ), f)
        B_m[:M, :] = (visc[None, :] == jm[:, None]).astype(f)
        blob = np.zeros((128, BLOB16), f)
        blob[:, O_CQ:O_CQ + NQ] = cq_full[:, ns]
        blob[:, O_CQ + NQ:O_CQ + NQT] = cq_full[:, ns]
        blob[:, O_SQ:O_SQ + NQ] = sq_full[:, ns]
        blob[:, O_SQ + NQ:O_SQ + NQT] = sq_full[:, ns]
        blob[:, O_CK:O_CK + M2] = ck2
        blob[:, O_SK:O_SK + M2] = sk2
        blob[:, O_A:O_A + M] = A_m
        blob[:, O_B:O_B + NQ] = B_m
        in_maps.append(dict(shared, xT=xTc, blob16=blob.astype(BF)))
    return in_maps, pbias_host


_NC_CACHE = {}


def _get_program():
    if "nc" not in _NC_CACHE:
        _NC_CACHE["nc"] = build_program()
    return _NC_CACHE["nc"]


def kernel(x, Wq, Wkv, sr_w, sr_b, ln_g, ln_b, proj_w, proj_b, H=None, W=None,
           _trace=False):
    nc = _get_program()
    in_maps, pbias_host = _host_prep(x, Wq, Wkv, sr_w, sr_b, ln_g, ln_b,
                                     proj_w, proj_b)
    res = run_bass_kernel_spmd(nc, in_maps, list(range(N_CORES)),
                               trace=_trace)
    kernel.last_result = res
    out = np.empty((B, N, C), np.float32)
    for core in range(N_CORES):
        y = np.asarray(res.results[core]["y"]).astype(np.float32)
        out[:, core * NQ:(core + 1) * NQ, :] = y.reshape(B, NQ, C)
    out += pbias_host[None, None, :]
    return out


# revision 16
# speedup vs baseline: 1.4715x; 1.0290x over previous
"""Trainium2 Bass kernel for nn_GSAttention (spatial-reduction attention).

Strategy (v1, bf16)
-------------------
* Queries sharded 512/core over 8 cores; each core handles both batches
  (1024 query rows).  KV path (conv+LN+KV proj) replicated per core but
  only for the 96 reduced tokens the causal mask can ever see.
* All matmul operands are bf16 (1 cyc/row on PE, half the HBM traffic of
  fp32); accumulation stays fp32 in PSUM.  Host pre-packs every operand
  in device layout; all small tables ride in one fused DMA blob.
* LN affine (g, b) is folded into the K/V projection weights on the host;
  the V-side bias collapses into the final projection bias, which is
  added on the host after the gather (it is exact: attention rows sum
  to 1).  K-side bias is applied on-device per-partition during the
  PSUM->SBUF copy.
* The spatial-causal mask is applied as a third matmul accumulated into
  the QK PSUM tile: lhsT = triangular NEG matrix A [96,96], rhs = onehot
  B[j,n] = [vis(n)==j], so z += A[vis(n),m] = NEG*[m>=vis(n)].
* Softmax normalization: V is augmented with a ones column (row 64 of
  the AV output = denominator); 1/den via DVE reciprocal [1,512] -> PE
  ones-matmul broadcast to [64,512] PSUM -> one tensor-mul (DVE/Pool
  alternating) writes the normalized, bf16 attention output.
"""

import os
import sys

for _p in ("/opt/trn_rl_repo", "/root/.axon_site/_ro/trn_rl_repo"):
    if os.path.isdir(_p) and _p not in sys.path:
        sys.path.insert(0, _p)

from contextlib import ExitStack

import numpy as np
import ml_dtypes

import types as _types
if "antenv.axon_hooks" not in sys.modules:
    _axh = _types.ModuleType("antenv.axon_hooks")
    _axh.get_axon_ntff_profile_hook = lambda: None
    sys.modules["antenv.axon_hooks"] = _axh

import concourse.bacc as bacc
import concourse.mybir as mybir
from concourse.tile import TileContext
from concourse.bass_utils import run_bass_kernel_spmd

F32 = mybir.dt.float32
BF16 = mybir.dt.bfloat16
AF = mybir.ActivationFunctionType
ALU = mybir.AluOpType
BF = ml_dtypes.bfloat16

# Problem constants (hardcoded).
N_CORES = 8
B = 2
N = 4096
C = 768
HEADS = 12
HD = 64
SR = 2
H = W = 64
NQ = 512            # queries per core per batch
NQT = B * NQ        # query rows per core
M = 96              # padded visible reduced tokens (real max vis = 94)
M2 = B * M
KC = C * SR * SR    # 3072 conv contraction
SCALE = 1.0 / 8.0
NEG = -60000.0
NCH = C // 128      # 6 feature chunks

# blob16 column offsets
O_CQ = 0
O_SQ = O_CQ + NQT
O_CK = O_SQ + NQT
O_SK = O_CK + M2
O_A = O_SK + M2
O_B = O_A + M
BLOB16 = O_B + NQ


def build_program():
    nc = bacc.Bacc("TRN2", target_bir_lowering=False, debug=False,
                   num_devices=N_CORES)

    def par(name, shape, out=False, dt=F32):
        return nc.declare_dram_parameter(name, list(shape), dt, isOutput=out)

    xT = par("xT", (C, NQT), dt=BF16)
    xi2c = par("xi2c", (KC, 256), dt=BF16)
    wqT = par("wqT", (C, C), dt=BF16)
    wkT = par("wkT", (C, C), dt=BF16)
    wvT = par("wvT", (C, C), dt=BF16)
    srwT = par("srwT", (KC, C), dt=BF16)
    projT = par("projT", (C, C), dt=BF16)
    blob16 = par("blob16", (128, BLOB16), dt=BF16)
    blob32 = par("blob32", (128, 12), dt=F32)
    Y = par("y", (NQT, C), out=True, dt=BF16)

    with TileContext(nc) as tc, ExitStack() as st:
        st.enter_context(nc.allow_low_precision(
            reason="bf16 reciprocal of softmax denominator; rel tol 2e-2"))
        pers = st.enter_context(tc.tile_pool(name="pers", bufs=1))

        # ---- persistent tiles -----------------------------------------
        b16 = pers.tile([128, BLOB16], BF16, tag="b16", name="b16")
        b32 = pers.tile([128, 12], F32, tag="b32", name="b32")
        rotq = [pers.tile([128, NQT], BF16, tag=f"rotq{i}", name=f"rotq{i}")
                for i in range(NCH)]
        rotk = [pers.tile([128, M2], BF16, tag=f"rotk{i}", name=f"rotk{i}")
                for i in range(NCH)]
        vaug = [pers.tile([M, HEADS * (HD + 1)], BF16, tag=f"vaug{b}",
                          name=f"vaug{b}") for b in range(B)]
        attnT = [pers.tile([128, NQT], BF16, tag=f"attnT{i}", name=f"attnT{i}")
                 for i in range(NCH)]
        xln = [pers.tile([128, M2], BF16, tag=f"xln{o}", name=f"xln{o}")
               for o in range(NCH)]
        ones_t = pers.tile([128, 1], BF16, tag="ones", name="ones")
        one1_t = pers.tile([1, 128], BF16, tag="one1", name="one1")
        eps_t = pers.tile([1, 1], F32, tag="eps", name="eps")

        nc.vector.memset(ones_t[:], 1.0)
        nc.vector.memset(one1_t[:], 1.0)
        nc.vector.memset(eps_t[:], 1e-5)

        cq_s = b16[:, O_CQ:O_CQ + NQT]
        sq_s = b16[:, O_SQ:O_SQ + NQT]
        ck_s = b16[:, O_CK:O_CK + M2]
        sk_s = b16[:, O_SK:O_SK + M2]
        A_s = b16[0:M, O_A:O_A + M]
        B_s = b16[0:M, O_B:O_B + NQ]

        with tc.tile_pool(name="pIn", bufs=1) as pIn, \
             tc.tile_pool(name="pSrw", bufs=4) as pSrw, \
             tc.tile_pool(name="pQs", bufs=1) as pQs, \
             tc.tile_pool(name="pT", bufs=1) as pT, \
             tc.tile_pool(name="pS", bufs=2) as pS, \
             tc.tile_pool(name="psQ", bufs=2, space="PSUM") as psQ, \
             tc.tile_pool(name="psA", bufs=1, space="PSUM") as psA:

            # ---- fused input DMAs (SP queue, bandwidth-ordered) -------
            def big_dma(tile, dram_ap, a, k):
                nc.sync.dma_start(
                    out=tile[:].rearrange("p (a k) -> p a k", a=a),
                    in_=dram_ap.rearrange("(a p) k -> p a k", p=128))

            wq_t = pIn.tile([128, NCH * C], BF16, tag="wq", name="wq")
            big_dma(wq_t, wqT[:], NCH, C)
            xT_t = pIn.tile([128, NCH * NQT], BF16, tag="xT", name="xT")
            big_dma(xT_t, xT[:], NCH, NQT)
            nc.sync.dma_start(out=b16[:], in_=blob16[:])
            nc.sync.dma_start(out=b32[:], in_=blob32[:])
            xi_t = pIn.tile([128, 24 * 256], BF16, tag="xi", name="xi")
            big_dma(xi_t, xi2c[:], 24, 256)
            srw_t = []
            for s in range(4):
                t = pSrw.tile([128, NCH * C], BF16, tag="srw", name=f"srw{s}")
                big_dma(t, srwT[s * 768:(s + 1) * 768, :], NCH, C)
                srw_t.append(t)
            wk_t = pIn.tile([128, NCH * C], BF16, tag="wk", name="wk")
            big_dma(wk_t, wkT[:], NCH, C)
            wv_t = pIn.tile([128, NCH * C], BF16, tag="wv", name="wv")
            big_dma(wv_t, wvT[:], NCH, C)
            pj_t = pIn.tile([128, NCH * C], BF16, tag="pj", name="pj")
            big_dma(pj_t, projT[:], NCH, C)

            # ---- Q projection + RoPE ----------------------------------
            q_sb = [pQs.tile([128, NQT], BF16, tag=f"qsb{o}", name=f"qsb{o}")
                    for o in range(NCH)]

            def q_block(o, nh):
                ns = slice(nh * NQ, (nh + 1) * NQ)
                q_ps = psQ.tile([128, NQ], F32, tag="q", name=f"q{o}{nh}")
                for cc in range(NCH):
                    nc.tensor.matmul(
                        q_ps[:], wq_t[:, cc * C + o * 128: cc * C + (o + 1) * 128],
                        xT_t[:, cc * NQT + nh * NQ: cc * NQT + (nh + 1) * NQ],
                        start=(cc == 0), stop=(cc == NCH - 1))
                nc.scalar.copy(q_sb[o][:, ns], q_ps[:])

            def rope_pair(c):
                t1 = pT.tile([128, NQT], BF16, tag="t1", name="t1")
                t2 = pT.tile([128, NQT], BF16, tag="t2", name="t2")
                nc.vector.tensor_mul(t1[:], q_sb[c][:], cq_s)
                nc.vector.tensor_mul(t2[:], q_sb[c + 3][:], sq_s)
                nc.vector.tensor_sub(rotq[c][:], t1[:], t2[:])
                t3 = pT.tile([128, NQT], BF16, tag="t3", name="t3")
                t4 = pT.tile([128, NQT], BF16, tag="t4", name="t4")
                nc.vector.tensor_mul(t3[:], q_sb[c + 3][:], cq_s)
                nc.vector.tensor_mul(t4[:], q_sb[c][:], sq_s)
                nc.vector.tensor_add(rotq[c + 3][:], t3[:], t4[:])

            for c in range(3):
                for o in (c, c + 3):
                    q_block(o, 0)
                    q_block(o, 1)
                rope_pair(c)

            # ---- conv (spatial reduction) -----------------------------
            xr_ps = [psA.tile([128, M2], F32, tag=f"xr{o}", name=f"xr{o}")
                     for o in range(NCH)]
            for kc in range(24):
                s, w = kc // 6, kc % 6
                xi_sl = xi_t[:, kc * 256: kc * 256 + M2]
                for o in range(NCH):
                    nc.tensor.matmul(
                        xr_ps[o][:],
                        srw_t[s][:, w * C + o * 128: w * C + (o + 1) * 128],
                        xi_sl, start=(kc == 0), stop=(kc == 23))

            # ---- LN (stats via ones-matmul; affine folded on host) ----
            xr_sb = [pS.tile([128, M2], BF16, tag=f"xrs{o}", name=f"xrs{o}")
                     for o in range(NCH)]
            for o in range(NCH):
                nc.scalar.activation(xr_sb[o][:], xr_ps[o][:], AF.Identity,
                                     bias=b32[:, o:o + 1])
            sum_ps = psA.tile([1, M2], F32, tag="xr0", name="sum")
            for o in range(NCH):
                nc.tensor.matmul(sum_ps[:], ones_t[:], xr_sb[o][:],
                                 start=(o == 0), stop=(o == NCH - 1))
            ssq_ps = psA.tile([1, M2], F32, tag="xr1", name="ssq")
            for o in range(NCH):
                sqt = pS.tile([128, M2], BF16, tag="sqt", name="sqt")
                nc.vector.tensor_mul(sqt[:], xr_sb[o][:], xr_sb[o][:])
                nc.tensor.matmul(ssq_ps[:], ones_t[:], sqt[:],
                                 start=(o == 0), stop=(o == NCH - 1))
            mu = pS.tile([1, M2], F32, tag="mu", name="mu")
            mu2 = pS.tile([1, M2], F32, tag="mu2", name="mu2")
            var = pS.tile([1, M2], F32, tag="var", name="var")
            std = pS.tile([1, M2], F32, tag="std", name="std")
            bsrc = pS.tile([1, 2 * M2], BF16, tag="bsrc", name="bsrc")
            nc.scalar.mul(mu[:], sum_ps[:], 1.0 / C)
            nc.vector.tensor_mul(mu2[:], mu[:], mu[:])
            nc.vector.scalar_tensor_tensor(var[:], ssq_ps[:], 1.0 / C, mu2[:],
                                           ALU.mult, ALU.subtract)
            nc.scalar.activation(std[:], var[:], AF.Sqrt, bias=eps_t[:])
            nc.vector.reciprocal(bsrc[:, 0:M2], std[:])
            nc.vector.scalar_tensor_tensor(bsrc[:, M2:2 * M2], mu[:], -1.0,
                                           bsrc[:, 0:M2], ALU.mult, ALU.mult)
            bc_ps = psA.tile([128, 2 * M2], F32, tag="xr2", name="bc")
            nc.tensor.matmul(bc_ps[:], one1_t[:], bsrc[:], start=True,
                             stop=True)
            bc_sb = pS.tile([128, 2 * M2], BF16, tag="bcs", name="bcs")
            nc.scalar.copy(bc_sb[:], bc_ps[:])
            for o in range(NCH):
                t = pS.tile([128, M2], BF16, tag="lnt", name="lnt")
                nc.vector.tensor_mul(t[:], xr_sb[o][:], bc_sb[:, 0:M2])
                nc.vector.tensor_add(xln[o][:], t[:], bc_sb[:, M2:2 * M2])

            # ---- K projection (+bias) + RoPE --------------------------
            k_ps = [psA.tile([128, M2], F32, tag=f"xr{o}", name=f"k{o}")
                    for o in range(NCH)]
            for o in range(NCH):
                for cc in range(NCH):
                    nc.tensor.matmul(
                        k_ps[o][:], wk_t[:, cc * C + o * 128: cc * C + (o + 1) * 128],
                        xln[cc][:], start=(cc == 0), stop=(cc == NCH - 1))
            k_sb = [pS.tile([128, M2], BF16, tag=f"ksb{o}", name=f"ksb{o}")
                    for o in range(NCH)]
            for o in range(NCH):
                nc.scalar.activation(k_sb[o][:], k_ps[o][:], AF.Identity,
                                     bias=b32[:, 6 + o:7 + o])
            for c in range(3):
                t1 = pS.tile([128, M2], BF16, tag="kt1", name="kt1")
                t2 = pS.tile([128, M2], BF16, tag="kt2", name="kt2")
                nc.vector.tensor_mul(t1[:], k_sb[c][:], ck_s)
                nc.vector.tensor_mul(t2[:], k_sb[c + 3][:], sk_s)
                nc.vector.tensor_sub(rotk[c][:], t1[:], t2[:])
                t3 = pS.tile([128, M2], BF16, tag="kt3", name="kt3")
                t4 = pS.tile([128, M2], BF16, tag="kt4", name="kt4")
                nc.vector.tensor_mul(t3[:], k_sb[c + 3][:], ck_s)
                nc.vector.tensor_mul(t4[:], k_sb[c][:], sk_s)
                nc.vector.tensor_add(rotk[c + 3][:], t3[:], t4[:])

            # ---- V projection into 65-col augmented layout ------------
            for b in range(B):
                for half in range(2):
                    v_ps = psA.tile([M, 384], F32, tag=f"xr{2 * b + half}",
                                    name=f"v{b}{half}")
                    for cc in range(NCH):
                        nc.tensor.matmul(
                            v_ps[:], xln[cc][:, b * M:(b + 1) * M],
                            wv_t[:, cc * C + half * 384: cc * C + (half + 1) * 384],
                            start=(cc == 0), stop=(cc == NCH - 1))
                    dst = vaug[b][:].rearrange("p (h d) -> p h d", d=HD + 1)
                    src_ = v_ps[:].rearrange("p (h d) -> p h d", d=HD)
                    nc.vector.tensor_copy(
                        dst[:, half * 6:(half + 1) * 6, 0:HD], src_)
                ocol = vaug[b][:].rearrange("p (h d) -> p h d", d=HD + 1)
                nc.vector.memset(ocol[:, :, HD:HD + 1], 1.0)

        # ================= Phase D: attention =========================
        # Per (batch, head): z = k^T q + mask (3 accumulated matmuls),
        # e = exp(z/8) [Act], u = V_aug^T e [PE] (row 64 = denominator),
        # rec = 1/den [DVE, bf16 SBUF].  Pairs of rec rows bounce through a
        # DRAM tile and return partition-broadcast to [64, 2*NQ] SBUF (DMA
        # engines are idle in this phase); one tensor-mul per unit
        # (DVE/Pool alternating) then writes normalized bf16 attnT.
        units = [(b, h) for b in range(B) for h in range(HEADS)]
        with tc.tile_pool(name="pD", bufs=3) as pD, \
             tc.tile_pool(name="pBc", bufs=3) as pBc, \
             tc.tile_pool(name="psD", bufs=1, space="PSUM") as psD:
            us, bcs = {}, {}
            for i in range(len(units) + 2):
                if i < len(units):
                    b, h = units[i]
                    hq, hr = h // 4, (h % 4) * 32
                    rs = slice(hr, hr + 32)
                    ms = slice(b * M, (b + 1) * M)
                    qs = slice(b * NQ, (b + 1) * NQ)
                    z_ps = psD.tile([M, NQ], F32, tag=f"z{i % 2}", name="z")
                    tp = (hr, 0) if hr == 96 else None
                    nc.tensor.matmul(z_ps[:], rotk[hq][rs, ms],
                                     rotq[hq][rs, qs], start=True,
                                     stop=False, tile_position=tp)
                    nc.tensor.matmul(z_ps[:], rotk[hq + 3][rs, ms],
                                     rotq[hq + 3][rs, qs], start=False,
                                     stop=False, tile_position=tp)
                    nc.tensor.matmul(z_ps[:], A_s, B_s, start=False,
                                     stop=True)
                    e_sb = pD.tile([M, NQ], BF16, tag="e", name="e")
                    nc.scalar.activation(e_sb[:], z_ps[:], AF.Exp,
                                         scale=SCALE)
                    u_ps = psD.tile([HD + 1, NQ], F32, tag=f"u{i % 4}",
                                    name="u")
                    nc.tensor.matmul(
                        u_ps[:],
                        vaug[b][:, h * (HD + 1):(h + 1) * (HD + 1)],
                        e_sb[:], start=True, stop=True)
                    rec = pD.tile([1, NQ], BF16, tag="rec", name="rec")
                    nc.vector.reciprocal(rec[:], u_ps[HD:HD + 1, :])
                    bc = pBc.tile([HD, NQ], BF16, tag="bc", name="bc")
                    nc.gpsimd.partition_broadcast(bc[:], rec[:])
                    us[i], bcs[i] = u_ps, bc
                if i >= 2:
                    j = i - 2
                    b, h = units[j]
                    dst = attnT[h // 2][(h % 2) * HD:(h % 2 + 1) * HD,
                                        b * NQ:(b + 1) * NQ]
                    nc.vector.tensor_mul(dst, us[j][0:HD, :], bcs[j][:])
                    del us[j], bcs[j]

        # ================= Phase E: output projection =================
        with tc.tile_pool(name="pE", bufs=4) as pE, \
             tc.tile_pool(name="psE", bufs=4, space="PSUM") as psE:
            for b in range(B):
                for nk in range(4):
                    r0 = b * NQ + nk * 128
                    y_sb = pE.tile([128, C], BF16, tag="y", name="y")
                    for ph in range(2):
                        y_ps = psE.tile([128, 384], F32, tag="yp", name="yp")
                        for oc in range(NCH):
                            nc.tensor.matmul(
                                y_ps[:], attnT[oc][:, r0:r0 + 128],
                                pj_t[:, oc * C + ph * 384: oc * C + (ph + 1) * 384],
                                start=(oc == 0), stop=(oc == NCH - 1))
                        if ph == 0:
                            nc.scalar.copy(y_sb[:, 0:384], y_ps[:])
                        else:
                            nc.vector.tensor_copy(y_sb[:, 384:768], y_ps[:])
                    nc.gpsimd.dma_start(out=Y[r0:r0 + 128, :], in_=y_sb[:])

    nc.compile()
    return nc


# ======================= host-side preparation =======================

def _angles(dim, end, w, step=1.0, bias=0.0, theta=10000.0):
    flat = np.arange(end, dtype=np.float32)
    xp = (bias + (flat % w) * step).astype(np.float32)
    yp = (bias + (flat // w) * step).astype(np.float32)
    freqs = (1.0 / theta ** (np.arange(0, dim, 4, dtype=np.float32)[: dim // 4]
                             / dim)).astype(np.float32)
    xf = np.outer(xp, freqs)
    yf = np.outer(yp, freqs)
    return np.stack([xf, yf], axis=-1).reshape(end, -1).astype(np.float32)


def _host_prep(x, Wq, Wkv, sr_w, sr_b, ln_g, ln_b, proj_w, proj_b):
    f = np.float32
    x = np.asarray(x, f)
    Wq = np.asarray(Wq, f)
    Wkv = np.asarray(Wkv, f)
    sr_w = np.asarray(sr_w, f)
    proj_w = np.asarray(proj_w, f)
    sr_b = np.asarray(sr_b, f)
    ln_g = np.asarray(ln_g, f)
    ln_b = np.asarray(ln_b, f)
    proj_b = np.asarray(proj_b, f)

    # head-dim permutation: pair-first -> rows 0..383, pair-second -> 384..767
    hh = np.arange(HEADS)[:, None] * HD
    jj = np.arange(HD // 2)[None, :] * 2
    perm = np.concatenate([(hh + jj).ravel(), (hh + jj + 1).ravel()])

    Wk = Wkv[:C]
    Wv = Wkv[C:]
    wqT = np.ascontiguousarray(Wq[perm, :].T.astype(BF))
    wkT = np.ascontiguousarray((Wk * ln_g[None, :])[perm, :].T.astype(BF))
    wvT = np.ascontiguousarray((Wv * ln_g[None, :]).T.astype(BF))
    kbias = (Wk @ ln_b)[perm].astype(f)
    vbias = (Wv @ ln_b).astype(f)
    pbias_host = (proj_b + proj_w @ vbias).astype(f)
    srwT = np.ascontiguousarray(sr_w.reshape(C, KC).T.astype(BF))
    projT = np.ascontiguousarray(proj_w.T.astype(BF))

    # im2col of the first 6 image rows, both batches: [3072, 192] pad 256
    strip = x[:, :6 * W, :].reshape(B, 3, 2, 32, 2, C)   # b, i, di, j, dj, c
    xi2c = np.zeros((KC, 256), f)
    xi2c[:, :M2] = strip.transpose(5, 2, 4, 0, 1, 3).reshape(KC, M2)
    xi2c = xi2c.astype(BF)

    # RoPE tables
    ang_q = _angles(HD, N, W)
    ang_k = _angles(HD, N // (SR * SR), W, step=SR, bias=1.0 - 1.0 / SR)
    rowj = np.arange(128) % 32
    cq_full = np.cos(ang_q)[:, rowj].T                   # [128, 4096]
    sq_full = np.sin(ang_q)[:, rowj].T
    ckk = np.cos(ang_k)[:M, rowj].T                      # [128, 96]
    skk = np.sin(ang_k)[:M, rowj].T
    ck2 = np.concatenate([ckk, ckk], 1)
    sk2 = np.concatenate([skk, skk], 1)

    # visibility
    n_all = np.arange(N)
    xpos = n_all // (SR * H)
    ox = n_all // H
    oy = n_all % H
    ypos = (ox + oy * H) // (SR * H)
    vis = xpos * SR + ypos + 1                            # [4096], 1..94

    # mask matrices: A[j, m] = NEG*[m >= j]; B[j, n] = [vis(n) == j]
    A_m = np.zeros((128, M), f)
    jm = np.arange(M)
    A_m[:M, :] = np.where(jm[None, :] >= jm[:, None], NEG, 0.0)

    blob32 = np.zeros((128, 12), f)
    blob32[:, 0:6] = sr_b.reshape(NCH, 128).T
    blob32[:, 6:12] = kbias.reshape(NCH, 128).T

    shared = dict(xi2c=xi2c, wqT=wqT, wkT=wkT, wvT=wvT, srwT=srwT,
                  projT=projT, blob32=blob32)

    in_maps = []
    for core in range(N_CORES):
        ns = slice(core * NQ, (core + 1) * NQ)
        xs = x[:, ns, :]
        xTc = np.ascontiguousarray(
            xs.transpose(2, 0, 1).reshape(C, NQT).astype(BF))
        visc = vis[ns]
        B_m = np.zeros((128, NQ), f)
        B_m[:M, :] = (visc[None, :] == jm[:, None]).astype(f)
        blob = np.zeros((128, BLOB16), f)
        blob[:, O_CQ:O_CQ + NQ] = cq_full[:, ns]
        blob[:, O_CQ + NQ:O_CQ + NQT] = cq_full[:, ns]
        blob[:, O_SQ:O_SQ + NQ] = sq_full[:, ns]
        blob[:, O_SQ + NQ:O_SQ + NQT] = sq_full[:, ns]
        blob[:, O_CK:O_CK + M2] = ck2
        blob[:, O_SK:O_SK + M2] = sk2
        blob[:, O_A:O_A + M] = A_m
        blob[:, O_B:O_B + NQ] = B_m
        in_maps.append(dict(shared, xT=xTc, blob16=blob.astype(BF)))
    return in_maps, pbias_host


_NC_CACHE = {}


def _get_program():
    if "nc" not in _NC_CACHE:
        _NC_CACHE["nc"] = build_program()
    return _NC_CACHE["nc"]


def kernel(x, Wq, Wkv, sr_w, sr_b, ln_g, ln_b, proj_w, proj_b, H=None, W=None,
           _trace=False):
    nc = _get_program()
    in_maps, pbias_host = _host_prep(x, Wq, Wkv, sr_w, sr_b, ln_g, ln_b,
                                     proj_w, proj_b)
    res = run_bass_kernel_spmd(nc, in_maps, list(range(N_CORES)),
                               trace=_trace)
    kernel.last_result = res
    out = np.empty((B, N, C), np.float32)
    for core in range(N_CORES):
        y = np.asarray(res.results[core]["y"]).astype(np.float32)
        out[:, core * NQ:(core + 1) * NQ, :] = y.reshape(B, NQ, C)
    out += pbias_host[None, None, :]
    return out


# revision 17
# speedup vs baseline: 1.4832x; 1.0080x over previous
"""Trainium2 Bass kernel for nn_GSAttention (spatial-reduction attention).

Strategy (v1, bf16)
-------------------
* Queries sharded 512/core over 8 cores; each core handles both batches
  (1024 query rows).  KV path (conv+LN+KV proj) replicated per core but
  only for the 96 reduced tokens the causal mask can ever see.
* All matmul operands are bf16 (1 cyc/row on PE, half the HBM traffic of
  fp32); accumulation stays fp32 in PSUM.  Host pre-packs every operand
  in device layout; all small tables ride in one fused DMA blob.
* LN affine (g, b) is folded into the K/V projection weights on the host;
  the V-side bias collapses into the final projection bias, which is
  added on the host after the gather (it is exact: attention rows sum
  to 1).  K-side bias is applied on-device per-partition during the
  PSUM->SBUF copy.
* The spatial-causal mask is applied as a third matmul accumulated into
  the QK PSUM tile: lhsT = triangular NEG matrix A [96,96], rhs = onehot
  B[j,n] = [vis(n)==j], so z += A[vis(n),m] = NEG*[m>=vis(n)].
* Softmax normalization: V is augmented with a ones column (row 64 of
  the AV output = denominator); 1/den via DVE reciprocal [1,512] -> PE
  ones-matmul broadcast to [64,512] PSUM -> one tensor-mul (DVE/Pool
  alternating) writes the normalized, bf16 attention output.
"""

import os
import sys

for _p in ("/opt/trn_rl_repo", "/root/.axon_site/_ro/trn_rl_repo"):
    if os.path.isdir(_p) and _p not in sys.path:
        sys.path.insert(0, _p)

from contextlib import ExitStack

import numpy as np
import ml_dtypes

import types as _types
if "antenv.axon_hooks" not in sys.modules:
    _axh = _types.ModuleType("antenv.axon_hooks")
    _axh.get_axon_ntff_profile_hook = lambda: None
    sys.modules["antenv.axon_hooks"] = _axh

import concourse.bacc as bacc
import concourse.mybir as mybir
from concourse.tile import TileContext
from concourse.bass_utils import run_bass_kernel_spmd

F32 = mybir.dt.float32
BF16 = mybir.dt.bfloat16
AF = mybir.ActivationFunctionType
ALU = mybir.AluOpType
BF = ml_dtypes.bfloat16

# Problem constants (hardcoded).
N_CORES = 8
B = 2
N = 4096
C = 768
HEADS = 12
HD = 64
SR = 2
H = W = 64
NQ = 512            # queries per core per batch
NQT = B * NQ        # query rows per core
M = 96              # padded visible reduced tokens (real max vis = 94)
M2 = B * M
KC = C * SR * SR    # 3072 conv contraction
SCALE = 1.0 / 8.0
NEG = -60000.0
NCH = C // 128      # 6 feature chunks

# blob16 column offsets
O_CQ = 0
O_SQ = O_CQ + NQT
O_CK = O_SQ + NQT
O_SK = O_CK + M2
O_A = O_SK + M2
O_B = O_A + M
BLOB16 = O_B + NQ


def build_program():
    nc = bacc.Bacc("TRN2", target_bir_lowering=False, debug=False,
                   num_devices=N_CORES)

    def par(name, shape, out=False, dt=F32):
        return nc.declare_dram_parameter(name, list(shape), dt, isOutput=out)

    xT = par("xT", (C, NQT), dt=BF16)
    xi2c = par("xi2c", (KC, 256), dt=BF16)
    wqT = par("wqT", (C, C), dt=BF16)
    wkT = par("wkT", (C, C), dt=BF16)
    wvT = par("wvT", (C, C), dt=BF16)
    srwT = par("srwT", (KC, C), dt=BF16)
    projT = par("projT", (C, C), dt=BF16)
    blob16 = par("blob16", (128, BLOB16), dt=BF16)
    blob32 = par("blob32", (128, 12), dt=F32)
    Y = par("y", (NQT, C), out=True, dt=BF16)

    with TileContext(nc) as tc, ExitStack() as st:
        st.enter_context(nc.allow_low_precision(
            reason="bf16 reciprocal of softmax denominator; rel tol 2e-2"))
        pers = st.enter_context(tc.tile_pool(name="pers", bufs=1))

        # ---- persistent tiles -----------------------------------------
        b16 = pers.tile([128, BLOB16], BF16, tag="b16", name="b16")
        b32 = pers.tile([128, 12], F32, tag="b32", name="b32")
        rotq = [pers.tile([128, NQT], BF16, tag=f"rotq{i}", name=f"rotq{i}")
                for i in range(NCH)]
        rotk = [pers.tile([128, M2], BF16, tag=f"rotk{i}", name=f"rotk{i}")
                for i in range(NCH)]
        vaug = [pers.tile([M, HEADS * (HD + 1)], BF16, tag=f"vaug{b}",
                          name=f"vaug{b}") for b in range(B)]
        attnT = [pers.tile([128, NQT], BF16, tag=f"attnT{i}", name=f"attnT{i}")
                 for i in range(NCH)]
        xln = [pers.tile([128, M2], BF16, tag=f"xln{o}", name=f"xln{o}")
               for o in range(NCH)]
        ones_t = pers.tile([128, 1], BF16, tag="ones", name="ones")
        one1_t = pers.tile([1, 128], BF16, tag="one1", name="one1")
        eps_t = pers.tile([1, 1], F32, tag="eps", name="eps")

        nc.vector.memset(ones_t[:], 1.0)
        nc.vector.memset(one1_t[:], 1.0)
        nc.vector.memset(eps_t[:], 1e-5)

        cq_s = b16[:, O_CQ:O_CQ + NQT]
        sq_s = b16[:, O_SQ:O_SQ + NQT]
        ck_s = b16[:, O_CK:O_CK + M2]
        sk_s = b16[:, O_SK:O_SK + M2]
        A_s = b16[0:M, O_A:O_A + M]
        B_s = b16[0:M, O_B:O_B + NQ]

        with tc.tile_pool(name="pIn", bufs=1) as pIn, \
             tc.tile_pool(name="pSrw", bufs=4) as pSrw, \
             tc.tile_pool(name="pQs", bufs=1) as pQs, \
             tc.tile_pool(name="pT", bufs=1) as pT, \
             tc.tile_pool(name="pS", bufs=2) as pS, \
             tc.tile_pool(name="psQ", bufs=2, space="PSUM") as psQ, \
             tc.tile_pool(name="psA", bufs=1, space="PSUM") as psA:

            # ---- fused input DMAs (SP queue, bandwidth-ordered) -------
            def big_dma(tile, dram_ap, a, k):
                nc.sync.dma_start(
                    out=tile[:].rearrange("p (a k) -> p a k", a=a),
                    in_=dram_ap.rearrange("(a p) k -> p a k", p=128))

            wq_t = pIn.tile([128, NCH * C], BF16, tag="wq", name="wq")
            big_dma(wq_t, wqT[:], NCH, C)
            xT_t = pIn.tile([128, NCH * NQT], BF16, tag="xT", name="xT")
            big_dma(xT_t, xT[:], NCH, NQT)
            nc.sync.dma_start(out=b16[:], in_=blob16[:])
            nc.sync.dma_start(out=b32[:], in_=blob32[:])
            xi_t = pIn.tile([128, 24 * 256], BF16, tag="xi", name="xi")
            big_dma(xi_t, xi2c[:], 24, 256)
            srw_t = []
            for s in range(4):
                t = pSrw.tile([128, NCH * C], BF16, tag="srw", name=f"srw{s}")
                big_dma(t, srwT[s * 768:(s + 1) * 768, :], NCH, C)
                srw_t.append(t)
            wk_t = pIn.tile([128, NCH * C], BF16, tag="wk", name="wk")
            big_dma(wk_t, wkT[:], NCH, C)
            wv_t = pIn.tile([128, NCH * C], BF16, tag="wv", name="wv")
            big_dma(wv_t, wvT[:], NCH, C)
            pj_t = pIn.tile([128, NCH * C], BF16, tag="pj", name="pj")
            big_dma(pj_t, projT[:], NCH, C)

            # ---- Q projection + RoPE ----------------------------------
            q_sb = [pQs.tile([128, NQT], BF16, tag=f"qsb{o}", name=f"qsb{o}")
                    for o in range(NCH)]

            def q_block(o, nh):
                ns = slice(nh * NQ, (nh + 1) * NQ)
                q_ps = psQ.tile([128, NQ], F32, tag="q", name=f"q{o}{nh}")
                for cc in range(NCH):
                    nc.tensor.matmul(
                        q_ps[:], wq_t[:, cc * C + o * 128: cc * C + (o + 1) * 128],
                        xT_t[:, cc * NQT + nh * NQ: cc * NQT + (nh + 1) * NQ],
                        start=(cc == 0), stop=(cc == NCH - 1))
                nc.scalar.copy(q_sb[o][:, ns], q_ps[:])

            def rope_pair(c):
                t1 = pT.tile([128, NQT], BF16, tag="t1", name="t1")
                t2 = pT.tile([128, NQT], BF16, tag="t2", name="t2")
                nc.vector.tensor_mul(t1[:], q_sb[c][:], cq_s)
                nc.vector.tensor_mul(t2[:], q_sb[c + 3][:], sq_s)
                nc.vector.tensor_sub(rotq[c][:], t1[:], t2[:])
                t3 = pT.tile([128, NQT], BF16, tag="t3", name="t3")
                t4 = pT.tile([128, NQT], BF16, tag="t4", name="t4")
                nc.vector.tensor_mul(t3[:], q_sb[c + 3][:], cq_s)
                nc.vector.tensor_mul(t4[:], q_sb[c][:], sq_s)
                nc.vector.tensor_add(rotq[c + 3][:], t3[:], t4[:])

            for c in range(3):
                for o in (c, c + 3):
                    q_block(o, 0)
                    q_block(o, 1)
                rope_pair(c)

            # ---- conv (spatial reduction) -----------------------------
            xr_ps = [psA.tile([128, M2], F32, tag=f"xr{o}", name=f"xr{o}")
                     for o in range(NCH)]
            for kc in range(24):
                s, w = kc // 6, kc % 6
                xi_sl = xi_t[:, kc * 256: kc * 256 + M2]
                for o in range(NCH):
                    nc.tensor.matmul(
                        xr_ps[o][:],
                        srw_t[s][:, w * C + o * 128: w * C + (o + 1) * 128],
                        xi_sl, start=(kc == 0), stop=(kc == 23))

            # ---- LN (stats via ones-matmul; affine folded on host) ----
            xr_sb = [pS.tile([128, M2], BF16, tag=f"xrs{o}", name=f"xrs{o}")
                     for o in range(NCH)]
            for o in range(NCH):
                nc.scalar.activation(xr_sb[o][:], xr_ps[o][:], AF.Identity,
                                     bias=b32[:, o:o + 1])
            sum_ps = psA.tile([1, M2], F32, tag="xr0", name="sum")
            for o in range(NCH):
                nc.tensor.matmul(sum_ps[:], ones_t[:], xr_sb[o][:],
                                 start=(o == 0), stop=(o == NCH - 1))
            ssq_ps = psA.tile([1, M2], F32, tag="xr1", name="ssq")
            for o in range(NCH):
                sqt = pS.tile([128, M2], BF16, tag="sqt", name="sqt")
                nc.vector.tensor_mul(sqt[:], xr_sb[o][:], xr_sb[o][:])
                nc.tensor.matmul(ssq_ps[:], ones_t[:], sqt[:],
                                 start=(o == 0), stop=(o == NCH - 1))
            mu = pS.tile([1, M2], F32, tag="mu", name="mu")
            mu2 = pS.tile([1, M2], F32, tag="mu2", name="mu2")
            var = pS.tile([1, M2], F32, tag="var", name="var")
            std = pS.tile([1, M2], F32, tag="std", name="std")
            bsrc = pS.tile([1, 2 * M2], BF16, tag="bsrc", name="bsrc")
            nc.scalar.mul(mu[:], sum_ps[:], 1.0 / C)
            nc.vector.tensor_mul(mu2[:], mu[:], mu[:])
            nc.vector.scalar_tensor_tensor(var[:], ssq_ps[:], 1.0 / C, mu2[:],
                                           ALU.mult, ALU.subtract)
            nc.scalar.activation(std[:], var[:], AF.Sqrt, bias=eps_t[:])
            nc.vector.reciprocal(bsrc[:, 0:M2], std[:])
            nc.vector.scalar_tensor_tensor(bsrc[:, M2:2 * M2], mu[:], -1.0,
                                           bsrc[:, 0:M2], ALU.mult, ALU.mult)
            bc_ps = psA.tile([128, 2 * M2], F32, tag="xr2", name="bc")
            nc.tensor.matmul(bc_ps[:], one1_t[:], bsrc[:], start=True,
                             stop=True)
            bc_sb = pS.tile([128, 2 * M2], BF16, tag="bcs", name="bcs")
            nc.scalar.copy(bc_sb[:], bc_ps[:])
            for o in range(NCH):
                t = pS.tile([128, M2], BF16, tag="lnt", name="lnt")
                nc.vector.tensor_mul(t[:], xr_sb[o][:], bc_sb[:, 0:M2])
                nc.vector.tensor_add(xln[o][:], t[:], bc_sb[:, M2:2 * M2])

            # ---- K projection (+bias) + RoPE --------------------------
            k_ps = [psA.tile([128, M2], F32, tag=f"xr{o}", name=f"k{o}")
                    for o in range(NCH)]
            for o in range(NCH):
                for cc in range(NCH):
                    nc.tensor.matmul(
                        k_ps[o][:], wk_t[:, cc * C + o * 128: cc * C + (o + 1) * 128],
                        xln[cc][:], start=(cc == 0), stop=(cc == NCH - 1))
            k_sb = [pS.tile([128, M2], BF16, tag=f"ksb{o}", name=f"ksb{o}")
                    for o in range(NCH)]
            for o in range(NCH):
                nc.scalar.activation(k_sb[o][:], k_ps[o][:], AF.Identity,
                                     bias=b32[:, 6 + o:7 + o])
            for c in range(3):
                t1 = pS.tile([128, M2], BF16, tag="kt1", name="kt1")
                t2 = pS.tile([128, M2], BF16, tag="kt2", name="kt2")
                nc.vector.tensor_mul(t1[:], k_sb[c][:], ck_s)
                nc.vector.tensor_mul(t2[:], k_sb[c + 3][:], sk_s)
                nc.vector.tensor_sub(rotk[c][:], t1[:], t2[:])
                t3 = pS.tile([128, M2], BF16, tag="kt3", name="kt3")
                t4 = pS.tile([128, M2], BF16, tag="kt4", name="kt4")
                nc.vector.tensor_mul(t3[:], k_sb[c + 3][:], ck_s)
                nc.vector.tensor_mul(t4[:], k_sb[c][:], sk_s)
                nc.vector.tensor_add(rotk[c + 3][:], t3[:], t4[:])

            # ---- V projection into 65-col augmented layout ------------
            for b in range(B):
                for half in range(2):
                    v_ps = psA.tile([M, 384], F32, tag=f"xr{2 * b + half}",
                                    name=f"v{b}{half}")
                    for cc in range(NCH):
                        nc.tensor.matmul(
                            v_ps[:], xln[cc][:, b * M:(b + 1) * M],
                            wv_t[:, cc * C + half * 384: cc * C + (half + 1) * 384],
                            start=(cc == 0), stop=(cc == NCH - 1))
                    dst = vaug[b][:].rearrange("p (h d) -> p h d", d=HD + 1)
                    src_ = v_ps[:].rearrange("p (h d) -> p h d", d=HD)
                    nc.vector.tensor_copy(
                        dst[:, half * 6:(half + 1) * 6, 0:HD], src_)
                ocol = vaug[b][:].rearrange("p (h d) -> p h d", d=HD + 1)
                nc.vector.memset(ocol[:, :, HD:HD + 1], 1.0)

        # ================= Phase D: attention =========================
        # Per (batch, head): z = k^T q + mask (3 accumulated matmuls),
        # e = exp(z/8) [Act], u = V_aug^T e [PE] (row 64 = denominator),
        # rec = 1/den [DVE, bf16 SBUF].  Pairs of rec rows bounce through a
        # DRAM tile and return partition-broadcast to [64, 2*NQ] SBUF (DMA
        # engines are idle in this phase); one tensor-mul per unit
        # (DVE/Pool alternating) then writes normalized bf16 attnT.
        units = [(b, h) for b in range(B) for h in range(HEADS)]
        with tc.tile_pool(name="pD", bufs=3) as pD, \
             tc.tile_pool(name="pBc", bufs=3) as pBc, \
             tc.tile_pool(name="psD", bufs=1, space="PSUM") as psD:
            # Software-pipelined: slot i issues QK/mask(i) then AV(i-1) on
            # the in-order PE queue, so the PE never waits on exp(i) and
            # stays continuously busy (full p-state).
            es, us, bcs = {}, {}, {}
            for i in range(len(units) + 3):
                if i < len(units):
                    b, h = units[i]
                    hq, hr = h // 4, (h % 4) * 32
                    rs = slice(hr, hr + 32)
                    ms = slice(b * M, (b + 1) * M)
                    qs = slice(b * NQ, (b + 1) * NQ)
                    z_ps = psD.tile([M, NQ], F32, tag=f"z{i % 2}", name="z")
                    tp = (hr, 0) if hr == 96 else None
                    nc.tensor.matmul(z_ps[:], rotk[hq][rs, ms],
                                     rotq[hq][rs, qs], start=True,
                                     stop=False, tile_position=tp)
                    nc.tensor.matmul(z_ps[:], rotk[hq + 3][rs, ms],
                                     rotq[hq + 3][rs, qs], start=False,
                                     stop=False, tile_position=tp)
                    nc.tensor.matmul(z_ps[:], A_s, B_s, start=False,
                                     stop=True)
                    e_sb = pD.tile([M, NQ], BF16, tag=f"e{i % 2}", name="e")
                    nc.scalar.activation(e_sb[:], z_ps[:], AF.Exp,
                                         scale=SCALE)
                    es[i] = e_sb
                if i >= 1 and i - 1 < len(units):
                    j = i - 1
                    b, h = units[j]
                    u_ps = psD.tile([HD + 1, NQ], F32, tag=f"u{j % 4}",
                                    name="u")
                    nc.tensor.matmul(
                        u_ps[:],
                        vaug[b][:, h * (HD + 1):(h + 1) * (HD + 1)],
                        es.pop(j)[:], start=True, stop=True)
                    rec = pD.tile([1, NQ], BF16, tag="rec", name="rec")
                    nc.vector.reciprocal(rec[:], u_ps[HD:HD + 1, :])
                    bc = pBc.tile([HD, NQ], BF16, tag="bc", name="bc")
                    nc.gpsimd.partition_broadcast(bc[:], rec[:])
                    us[j], bcs[j] = u_ps, bc
                if i >= 3:
                    j = i - 3
                    b, h = units[j]
                    dst = attnT[h // 2][(h % 2) * HD:(h % 2 + 1) * HD,
                                        b * NQ:(b + 1) * NQ]
                    nc.vector.tensor_mul(dst, us[j][0:HD, :], bcs[j][:])
                    del us[j], bcs[j]

        # ================= Phase E: output projection =================
        with tc.tile_pool(name="pE", bufs=4) as pE, \
             tc.tile_pool(name="psE", bufs=4, space="PSUM") as psE:
            for b in range(B):
                for nk in range(4):
                    r0 = b * NQ + nk * 128
                    y_sb = pE.tile([128, C], BF16, tag="y", name="y")
                    for ph in range(2):
                        y_ps = psE.tile([128, 384], F32, tag="yp", name="yp")
                        for oc in range(NCH):
                            nc.tensor.matmul(
                                y_ps[:], attnT[oc][:, r0:r0 + 128],
                                pj_t[:, oc * C + ph * 384: oc * C + (ph + 1) * 384],
                                start=(oc == 0), stop=(oc == NCH - 1))
                        if ph == 0:
                            nc.scalar.copy(y_sb[:, 0:384], y_ps[:])
                        else:
                            nc.vector.tensor_copy(y_sb[:, 384:768], y_ps[:])
                    nc.gpsimd.dma_start(out=Y[r0:r0 + 128, :], in_=y_sb[:])

    nc.compile()
    return nc


# ======================= host-side preparation =======================

def _angles(dim, end, w, step=1.0, bias=0.0, theta=10000.0):
    flat = np.arange(end, dtype=np.float32)
    xp = (bias + (flat % w) * step).astype(np.float32)
    yp = (bias + (flat // w) * step).astype(np.float32)
    freqs = (1.0 / theta ** (np.arange(0, dim, 4, dtype=np.float32)[: dim // 4]
                             / dim)).astype(np.float32)
    xf = np.outer(xp, freqs)
    yf = np.outer(yp, freqs)
    return np.stack([xf, yf], axis=-1).reshape(end, -1).astype(np.float32)


def _host_prep(x, Wq, Wkv, sr_w, sr_b, ln_g, ln_b, proj_w, proj_b):
    f = np.float32
    x = np.asarray(x, f)
    Wq = np.asarray(Wq, f)
    Wkv = np.asarray(Wkv, f)
    sr_w = np.asarray(sr_w, f)
    proj_w = np.asarray(proj_w, f)
    sr_b = np.asarray(sr_b, f)
    ln_g = np.asarray(ln_g, f)
    ln_b = np.asarray(ln_b, f)
    proj_b = np.asarray(proj_b, f)

    # head-dim permutation: pair-first -> rows 0..383, pair-second -> 384..767
    hh = np.arange(HEADS)[:, None] * HD
    jj = np.arange(HD // 2)[None, :] * 2
    perm = np.concatenate([(hh + jj).ravel(), (hh + jj + 1).ravel()])

    Wk = Wkv[:C]
    Wv = Wkv[C:]
    wqT = np.ascontiguousarray(Wq[perm, :].T.astype(BF))
    wkT = np.ascontiguousarray((Wk * ln_g[None, :])[perm, :].T.astype(BF))
    wvT = np.ascontiguousarray((Wv * ln_g[None, :]).T.astype(BF))
    kbias = (Wk @ ln_b)[perm].astype(f)
    vbias = (Wv @ ln_b).astype(f)
    pbias_host = (proj_b + proj_w @ vbias).astype(f)
    srwT = np.ascontiguousarray(sr_w.reshape(C, KC).T.astype(BF))
    projT = np.ascontiguousarray(proj_w.T.astype(BF))

    # im2col of the first 6 image rows, both batches: [3072, 192] pad 256
    strip = x[:, :6 * W, :].reshape(B, 3, 2, 32, 2, C)   # b, i, di, j, dj, c
    xi2c = np.zeros((KC, 256), f)
    xi2c[:, :M2] = strip.transpose(5, 2, 4, 0, 1, 3).reshape(KC, M2)
    xi2c = xi2c.astype(BF)

    # RoPE tables
    ang_q = _angles(HD, N, W)
    ang_k = _angles(HD, N // (SR * SR), W, step=SR, bias=1.0 - 1.0 / SR)
    rowj = np.arange(128) % 32
    cq_full = np.cos(ang_q)[:, rowj].T                   # [128, 4096]
    sq_full = np.sin(ang_q)[:, rowj].T
    ckk = np.cos(ang_k)[:M, rowj].T                      # [128, 96]
    skk = np.sin(ang_k)[:M, rowj].T
    ck2 = np.concatenate([ckk, ckk], 1)
    sk2 = np.concatenate([skk, skk], 1)

    # visibility
    n_all = np.arange(N)
    xpos = n_all // (SR * H)
    ox = n_all // H
    oy = n_all % H
    ypos = (ox + oy * H) // (SR * H)
    vis = xpos * SR + ypos + 1                            # [4096], 1..94

    # mask matrices: A[j, m] = NEG*[m >= j]; B[j, n] = [vis(n) == j]
    A_m = np.zeros((128, M), f)
    jm = np.arange(M)
    A_m[:M, :] = np.where(jm[None, :] >= jm[:, None], NEG, 0.0)

    blob32 = np.zeros((128, 12), f)
    blob32[:, 0:6] = sr_b.reshape(NCH, 128).T
    blob32[:, 6:12] = kbias.reshape(NCH, 128).T

    shared = dict(xi2c=xi2c, wqT=wqT, wkT=wkT, wvT=wvT, srwT=srwT,
                  projT=projT, blob32=blob32)

    in_maps = []
    for core in range(N_CORES):
        ns = slice(core * NQ, (core + 1) * NQ)
        xs = x[:, ns, :]
        xTc = np.ascontiguousarray(
            xs.transpose(2, 0, 1).reshape(C, NQT).astype(BF))
        visc = vis[ns]
        B_m = np.zeros((128, NQ), f)
        B_m[:M, :] = (visc[None, :] == jm[:, None]).astype(f)
        blob = np.zeros((128, BLOB16), f)
        blob[:, O_CQ:O_CQ + NQ] = cq_full[:, ns]
        blob[:, O_CQ + NQ:O_CQ + NQT] = cq_full[:, ns]
        blob[:, O_SQ:O_SQ + NQ] = sq_full[:, ns]
        blob[:, O_SQ + NQ:O_SQ + NQT] = sq_full[:, ns]
        blob[:, O_CK:O_CK + M2] = ck2
        blob[:, O_SK:O_SK + M2] = sk2
        blob[:, O_A:O_A + M] = A_m
        blob[:, O_B:O_B + NQ] = B_m
        in_maps.append(dict(shared, xT=xTc, blob16=blob.astype(BF)))
    return in_maps, pbias_host


_NC_CACHE = {}


def _get_program():
    if "nc" not in _NC_CACHE:
        _NC_CACHE["nc"] = build_program()
    return _NC_CACHE["nc"]


def kernel(x, Wq, Wkv, sr_w, sr_b, ln_g, ln_b, proj_w, proj_b, H=None, W=None,
           _trace=False):
    nc = _get_program()
    in_maps, pbias_host = _host_prep(x, Wq, Wkv, sr_w, sr_b, ln_g, ln_b,
                                     proj_w, proj_b)
    res = run_bass_kernel_spmd(nc, in_maps, list(range(N_CORES)),
                               trace=_trace)
    kernel.last_result = res
    out = np.empty((B, N, C), np.float32)
    for core in range(N_CORES):
        y = np.asarray(res.results[core]["y"]).astype(np.float32)
        out[:, core * NQ:(core + 1) * NQ, :] = y.reshape(B, NQ, C)
    out += pbias_host[None, None, :]
    return out


# revision 22
# speedup vs baseline: 1.7304x; 1.1666x over previous
"""Trainium2 Bass kernel for nn_GSAttention (spatial-reduction attention).

Strategy (v1, bf16)
-------------------
* Queries sharded 512/core over 8 cores; each core handles both batches
  (1024 query rows).  KV path (conv+LN+KV proj) replicated per core but
  only for the 96 reduced tokens the causal mask can ever see.
* All matmul operands are bf16 (1 cyc/row on PE, half the HBM traffic of
  fp32); accumulation stays fp32 in PSUM.  Host pre-packs every operand
  in device layout; all small tables ride in one fused DMA blob.
* LN affine (g, b) is folded into the K/V projection weights on the host;
  the V-side bias collapses into the final projection bias, which is
  added on the host after the gather (it is exact: attention rows sum
  to 1).  K-side bias is applied on-device per-partition during the
  PSUM->SBUF copy.
* The spatial-causal mask is applied as a third matmul accumulated into
  the QK PSUM tile: lhsT = triangular NEG matrix A [96,96], rhs = onehot
  B[j,n] = [vis(n)==j], so z += A[vis(n),m] = NEG*[m>=vis(n)].
* Softmax normalization: V is augmented with a ones column (row 64 of
  the AV output = denominator); 1/den via DVE reciprocal [1,512] -> PE
  ones-matmul broadcast to [64,512] PSUM -> one tensor-mul (DVE/Pool
  alternating) writes the normalized, bf16 attention output.
"""

import os
import sys

for _p in ("/opt/trn_rl_repo", "/root/.axon_site/_ro/trn_rl_repo"):
    if os.path.isdir(_p) and _p not in sys.path:
        sys.path.insert(0, _p)

from contextlib import ExitStack

import numpy as np
import ml_dtypes

import types as _types
if "antenv.axon_hooks" not in sys.modules:
    _axh = _types.ModuleType("antenv.axon_hooks")
    _axh.get_axon_ntff_profile_hook = lambda: None
    sys.modules["antenv.axon_hooks"] = _axh

import concourse.bacc as bacc
import concourse.mybir as mybir
from concourse.tile import TileContext
from concourse.bass_utils import run_bass_kernel_spmd

F32 = mybir.dt.float32
BF16 = mybir.dt.bfloat16
AF = mybir.ActivationFunctionType
ALU = mybir.AluOpType
BF = ml_dtypes.bfloat16

# Problem constants (hardcoded).
N_CORES = 8
B = 2
N = 4096
C = 768
HEADS = 12
HD = 64
SR = 2
H = W = 64
NQ = 512            # queries per core per batch
NQT = B * NQ        # query rows per core
M = 96              # padded visible reduced tokens (real max vis = 94)
M2 = B * M
KC = C * SR * SR    # 3072 conv contraction
SCALE = 1.0 / 8.0
NEG = -60000.0
NCH = C // 128      # 6 feature chunks

# blob16 column offsets
O_CQ = 0
O_SQ = O_CQ + NQT
O_CK = O_SQ + NQT
O_SK = O_CK + M2
O_A = O_SK + M2
O_B = O_A + M
BLOB16 = O_B + NQ


def build_program():
    nc = bacc.Bacc("TRN2", target_bir_lowering=False, debug=False,
                   num_devices=N_CORES)

    def par(name, shape, out=False, dt=F32):
        return nc.declare_dram_parameter(name, list(shape), dt, isOutput=out)

    xT = par("xT", (C, NQT), dt=BF16)
    xi2c = par("xi2c", (KC, 256), dt=BF16)
    wqT = par("wqT", (C, C), dt=BF16)
    wkT = par("wkT", (C, C), dt=BF16)
    wvT = par("wvT", (C, C), dt=BF16)
    srwT = par("srwT", (KC, C), dt=BF16)
    projT = par("projT", (C, C), dt=BF16)
    blob16 = par("blob16", (128, BLOB16), dt=BF16)
    blob32 = par("blob32", (128, 12), dt=F32)
    Y = par("y", (NQT, C), out=True, dt=BF16)

    with TileContext(nc) as tc, ExitStack() as st:
        st.enter_context(nc.allow_low_precision(
            reason="bf16 reciprocal of softmax denominator; rel tol 2e-2"))
        pers = st.enter_context(tc.tile_pool(name="pers", bufs=1))

        # ---- persistent tiles -----------------------------------------
        b16 = pers.tile([128, BLOB16], BF16, tag="b16", name="b16")
        b32 = pers.tile([128, 12], F32, tag="b32", name="b32")
        rotq = [pers.tile([128, NQT], BF16, tag=f"rotq{i}", name=f"rotq{i}")
                for i in range(NCH)]
        rotk = [pers.tile([128, M2], BF16, tag=f"rotk{i}", name=f"rotk{i}")
                for i in range(NCH)]
        vaug = [pers.tile([M, HEADS * (HD + 1)], BF16, tag=f"vaug{b}",
                          name=f"vaug{b}") for b in range(B)]
        attnT = [pers.tile([128, NQT], BF16, tag=f"attnT{i}", name=f"attnT{i}")
                 for i in range(NCH)]
        xln = [pers.tile([128, M2], BF16, tag=f"xln{o}", name=f"xln{o}")
               for o in range(NCH)]
        ones_t = pers.tile([128, 1], BF16, tag="ones", name="ones")
        one1_t = pers.tile([1, 128], BF16, tag="one1", name="one1")
        eps_t = pers.tile([1, 1], F32, tag="eps", name="eps")
        pj_t = pers.tile([128, NCH * C], BF16, tag="pj", name="pj")

        nc.vector.memset(ones_t[:], 1.0)
        nc.vector.memset(one1_t[:], 1.0)
        nc.vector.memset(eps_t[:], 1e-5)

        cq_s = b16[:, O_CQ:O_CQ + NQT]
        sq_s = b16[:, O_SQ:O_SQ + NQT]
        ck_s = b16[:, O_CK:O_CK + M2]
        sk_s = b16[:, O_SK:O_SK + M2]
        A_s = b16[0:M, O_A:O_A + M]
        B_s = b16[0:M, O_B:O_B + NQ]

        with tc.tile_pool(name="pIn", bufs=1) as pIn, \
             tc.tile_pool(name="pSrw", bufs=4) as pSrw, \
             tc.tile_pool(name="pQs", bufs=1) as pQs, \
             tc.tile_pool(name="pT", bufs=1) as pT, \
             tc.tile_pool(name="pS", bufs=2) as pS, \
             tc.tile_pool(name="psQ", bufs=2, space="PSUM") as psQ, \
             tc.tile_pool(name="psA", bufs=1, space="PSUM") as psA:

            # ---- fused input DMAs (SP queue, bandwidth-ordered) -------
            def big_dma(tile, dram_ap, a, k):
                nc.sync.dma_start(
                    out=tile[:].rearrange("p (a k) -> p a k", a=a),
                    in_=dram_ap.rearrange("(a p) k -> p a k", p=128))

            wq_t = pIn.tile([128, NCH * C], BF16, tag="wq", name="wq")
            big_dma(wq_t, wqT[:], NCH, C)
            xT_t = pIn.tile([128, NCH * NQT], BF16, tag="xT", name="xT")
            big_dma(xT_t, xT[:], NCH, NQT)
            nc.sync.dma_start(out=b16[:], in_=blob16[:])
            nc.sync.dma_start(out=b32[:], in_=blob32[:])
            xi_t = pIn.tile([128, 24 * 256], BF16, tag="xi", name="xi")
            big_dma(xi_t, xi2c[:], 24, 256)
            srw_t = []
            for s in range(4):
                t = pSrw.tile([128, NCH * C], BF16, tag="srw", name=f"srw{s}")
                big_dma(t, srwT[s * 768:(s + 1) * 768, :], NCH, C)
                srw_t.append(t)
            wk_t = pIn.tile([128, NCH * C], BF16, tag="wk", name="wk")
            big_dma(wk_t, wkT[:], NCH, C)
            wv_t = pIn.tile([128, NCH * C], BF16, tag="wv", name="wv")
            big_dma(wv_t, wvT[:], NCH, C)
            big_dma(pj_t, projT[:], NCH, C)

            # ---- Q projection + RoPE ----------------------------------
            q_sb = [pQs.tile([128, NQT], BF16, tag=f"qsb{o}", name=f"qsb{o}")
                    for o in range(NCH)]

            def q_block(o, nh):
                ns = slice(nh * NQ, (nh + 1) * NQ)
                q_ps = psQ.tile([128, NQ], F32, tag="q", name=f"q{o}{nh}")
                for cc in range(NCH):
                    nc.tensor.matmul(
                        q_ps[:], wq_t[:, cc * C + o * 128: cc * C + (o + 1) * 128],
                        xT_t[:, cc * NQT + nh * NQ: cc * NQT + (nh + 1) * NQ],
                        start=(cc == 0), stop=(cc == NCH - 1))
                nc.scalar.copy(q_sb[o][:, ns], q_ps[:])

            def rope_pair(c):
                t1 = pT.tile([128, NQT], BF16, tag="t1", name="t1")
                t2 = pT.tile([128, NQT], BF16, tag="t2", name="t2")
                nc.vector.tensor_mul(t1[:], q_sb[c][:], cq_s)
                nc.vector.tensor_mul(t2[:], q_sb[c + 3][:], sq_s)
                nc.vector.tensor_sub(rotq[c][:], t1[:], t2[:])
                t3 = pT.tile([128, NQT], BF16, tag="t3", name="t3")
                t4 = pT.tile([128, NQT], BF16, tag="t4", name="t4")
                nc.vector.tensor_mul(t3[:], q_sb[c + 3][:], cq_s)
                nc.vector.tensor_mul(t4[:], q_sb[c][:], sq_s)
                nc.vector.tensor_add(rotq[c + 3][:], t3[:], t4[:])

            for c in range(3):
                for o in (c, c + 3):
                    q_block(o, 0)
                    q_block(o, 1)
                rope_pair(c)

            # ---- conv (spatial reduction) -----------------------------
            xr_ps = [psA.tile([128, M2], F32, tag=f"xr{o}", name=f"xr{o}")
                     for o in range(NCH)]
            for kc in range(24):
                s, w = kc // 6, kc % 6
                xi_sl = xi_t[:, kc * 256: kc * 256 + M2]
                for o in range(NCH):
                    nc.tensor.matmul(
                        xr_ps[o][:],
                        srw_t[s][:, w * C + o * 128: w * C + (o + 1) * 128],
                        xi_sl, start=(kc == 0), stop=(kc == 23))

            # ---- LN (stats via ones-matmul; affine folded on host) ----
            xr_sb = [pS.tile([128, M2], BF16, tag=f"xrs{o}", name=f"xrs{o}")
                     for o in range(NCH)]
            for o in range(NCH):
                nc.scalar.activation(xr_sb[o][:], xr_ps[o][:], AF.Identity,
                                     bias=b32[:, o:o + 1])
            sum_ps = psA.tile([1, M2], F32, tag="xr0", name="sum")
            for o in range(NCH):
                nc.tensor.matmul(sum_ps[:], ones_t[:], xr_sb[o][:],
                                 start=(o == 0), stop=(o == NCH - 1))
            ssq_ps = psA.tile([1, M2], F32, tag="xr1", name="ssq")
            for o in range(NCH):
                sqt = pS.tile([128, M2], BF16, tag="sqt", name="sqt")
                nc.vector.tensor_mul(sqt[:], xr_sb[o][:], xr_sb[o][:])
                nc.tensor.matmul(ssq_ps[:], ones_t[:], sqt[:],
                                 start=(o == 0), stop=(o == NCH - 1))
            mu = pS.tile([1, M2], F32, tag="mu", name="mu")
            mu2 = pS.tile([1, M2], F32, tag="mu2", name="mu2")
            var = pS.tile([1, M2], F32, tag="var", name="var")
            std = pS.tile([1, M2], F32, tag="std", name="std")
            bsrc = pS.tile([1, 2 * M2], BF16, tag="bsrc", name="bsrc")
            nc.scalar.mul(mu[:], sum_ps[:], 1.0 / C)
            nc.vector.tensor_mul(mu2[:], mu[:], mu[:])
            nc.vector.scalar_tensor_tensor(var[:], ssq_ps[:], 1.0 / C, mu2[:],
                                           ALU.mult, ALU.subtract)
            nc.scalar.activation(std[:], var[:], AF.Sqrt, bias=eps_t[:])
            nc.vector.reciprocal(bsrc[:, 0:M2], std[:])
            nc.vector.scalar_tensor_tensor(bsrc[:, M2:2 * M2], mu[:], -1.0,
                                           bsrc[:, 0:M2], ALU.mult, ALU.mult)
            bc_ps = psA.tile([128, 2 * M2], F32, tag="xr2", name="bc")
            nc.tensor.matmul(bc_ps[:], one1_t[:], bsrc[:], start=True,
                             stop=True)
            bc_sb = pS.tile([128, 2 * M2], BF16, tag="bcs", name="bcs")
            nc.scalar.copy(bc_sb[:], bc_ps[:])
            for o in range(NCH):
                t = pS.tile([128, M2], BF16, tag="lnt", name="lnt")
                nc.vector.tensor_mul(t[:], xr_sb[o][:], bc_sb[:, 0:M2])
                nc.vector.tensor_add(xln[o][:], t[:], bc_sb[:, M2:2 * M2])

            # ---- K projection (+bias) + RoPE --------------------------
            k_ps = [psA.tile([128, M2], F32, tag=f"xr{o}", name=f"k{o}")
                    for o in range(NCH)]
            for o in range(NCH):
                for cc in range(NCH):
                    nc.tensor.matmul(
                        k_ps[o][:], wk_t[:, cc * C + o * 128: cc * C + (o + 1) * 128],
                        xln[cc][:], start=(cc == 0), stop=(cc == NCH - 1))
            k_sb = [pS.tile([128, M2], BF16, tag=f"ksb{o}", name=f"ksb{o}")
                    for o in range(NCH)]
            for o in range(NCH):
                nc.scalar.activation(k_sb[o][:], k_ps[o][:], AF.Identity,
                                     bias=b32[:, 6 + o:7 + o])
            for c in range(3):
                t1 = pS.tile([128, M2], BF16, tag="kt1", name="kt1")
                t2 = pS.tile([128, M2], BF16, tag="kt2", name="kt2")
                nc.vector.tensor_mul(t1[:], k_sb[c][:], ck_s)
                nc.vector.tensor_mul(t2[:], k_sb[c + 3][:], sk_s)
                nc.vector.tensor_sub(rotk[c][:], t1[:], t2[:])
                t3 = pS.tile([128, M2], BF16, tag="kt3", name="kt3")
                t4 = pS.tile([128, M2], BF16, tag="kt4", name="kt4")
                nc.vector.tensor_mul(t3[:], k_sb[c + 3][:], ck_s)
                nc.vector.tensor_mul(t4[:], k_sb[c][:], sk_s)
                nc.vector.tensor_add(rotk[c + 3][:], t3[:], t4[:])

            # ---- V projection into 65-col augmented layout ------------
            for b in range(B):
                for half in range(2):
                    v_ps = psA.tile([M, 384], F32, tag=f"xr{2 * b + half}",
                                    name=f"v{b}{half}")
                    for cc in range(NCH):
                        nc.tensor.matmul(
                            v_ps[:], xln[cc][:, b * M:(b + 1) * M],
                            wv_t[:, cc * C + half * 384: cc * C + (half + 1) * 384],
                            start=(cc == 0), stop=(cc == NCH - 1))
                    dst = vaug[b][:].rearrange("p (h d) -> p h d", d=HD + 1)
                    src_ = v_ps[:].rearrange("p (h d) -> p h d", d=HD)
                    nc.vector.tensor_copy(
                        dst[:, half * 6:(half + 1) * 6, 0:HD], src_)
                ocol = vaug[b][:].rearrange("p (h d) -> p h d", d=HD + 1)
                nc.vector.memset(ocol[:, :, HD:HD + 1], 1.0)

        # ================= Phase D: attention =========================
        # Per (batch, head): z = k^T q + mask (3 accumulated matmuls),
        # e = exp(z/8) [Act], u = V_aug^T e [PE] (row 64 = denominator),
        # rec = 1/den [DVE, bf16 SBUF].  Pairs of rec rows bounce through a
        # DRAM tile and return partition-broadcast to [64, 2*NQ] SBUF (DMA
        # engines are idle in this phase); one tensor-mul per unit
        # (DVE/Pool alternating) then writes normalized bf16 attnT.
        units = [(b, h) for b in range(B) for h in range(HEADS)]
        with tc.tile_pool(name="pD", bufs=3) as pD, \
             tc.tile_pool(name="pBc", bufs=3) as pBc, \
             tc.tile_pool(name="pE", bufs=3) as pE, \
             tc.tile_pool(name="psD", bufs=1, space="PSUM") as psD, \
             tc.tile_pool(name="psE", bufs=2, space="PSUM") as psE:

            def e_block(b, nk):
                r0 = b * NQ + nk * 128
                y_sb = pE.tile([128, C], BF16, tag="y", name="y")
                for ph in range(2):
                    y_ps = psE.tile([128, 384], F32, tag="yp", name="yp")
                    for oc in range(NCH):
                        nc.tensor.matmul(
                            y_ps[:], attnT[oc][:, r0:r0 + 128],
                            pj_t[:, oc * C + ph * 384: oc * C + (ph + 1) * 384],
                            start=(oc == 0), stop=(oc == NCH - 1))
                    nc.scalar.copy(y_sb[:, ph * 384:(ph + 1) * 384], y_ps[:])
                nc.sync.dma_start(out=Y[r0:r0 + 128, :], in_=y_sb[:])
            # Software-pipelined: slot i issues QK/mask(i) then AV(i-1) on
            # the in-order PE queue, so the PE never waits on exp(i) and
            # stays continuously busy (full p-state).
            es, us, bcs = {}, {}, {}
            for i in range(len(units) + 3):
                if i < len(units):
                    b, h = units[i]
                    hq, hr = h // 4, (h % 4) * 32
                    rs = slice(hr, hr + 32)
                    ms = slice(b * M, (b + 1) * M)
                    qs = slice(b * NQ, (b + 1) * NQ)
                    z_ps = psD.tile([M, NQ], F32, tag=f"z{i % 2}", name="z")
                    tp = (hr, 0) if hr == 96 else None
                    nc.tensor.matmul(z_ps[:], rotk[hq][rs, ms],
                                     rotq[hq][rs, qs], start=True,
                                     stop=False, tile_position=tp)
                    nc.tensor.matmul(z_ps[:], rotk[hq + 3][rs, ms],
                                     rotq[hq + 3][rs, qs], start=False,
                                     stop=False, tile_position=tp)
                    nc.tensor.matmul(z_ps[:], A_s, B_s, start=False,
                                     stop=True)
                    e_sb = pD.tile([M, NQ], BF16, tag=f"e{i % 2}", name="e")
                    nc.scalar.activation(e_sb[:], z_ps[:], AF.Exp,
                                         scale=SCALE)
                    es[i] = e_sb
                if i >= 1 and i - 1 < len(units):
                    j = i - 1
                    b, h = units[j]
                    u_ps = psD.tile([HD + 1, NQ], F32, tag=f"u{j % 4}",
                                    name="u")
                    nc.tensor.matmul(
                        u_ps[:],
                        vaug[b][:, h * (HD + 1):(h + 1) * (HD + 1)],
                        es.pop(j)[:], start=True, stop=True)
                    rec = pD.tile([1, NQ], BF16, tag="rec", name="rec")
                    nc.vector.reciprocal(rec[:], u_ps[HD:HD + 1, :])
                    bc = pBc.tile([HD, NQ], BF16, tag="bc", name="bc")
                    nc.gpsimd.partition_broadcast(bc[:], rec[:])
                    us[j], bcs[j] = u_ps, bc
                if i >= 3:
                    j = i - 3
                    b, h = units[j]
                    dst = attnT[h // 2][(h % 2) * HD:(h % 2 + 1) * HD,
                                        b * NQ:(b + 1) * NQ]
                    nc.vector.tensor_mul(dst, us[j][0:HD, :], bcs[j][:])
                    del us[j], bcs[j]
                # E(b=0) blocks interleaved into D(b=1) to keep PE dense
                if i in (15, 18, 21, 24):
                    e_block(0, (i - 15) // 3)
            for nk in range(4):
                e_block(1, nk)

    nc.compile()
    return nc


# ======================= host-side preparation =======================

def _angles(dim, end, w, step=1.0, bias=0.0, theta=10000.0):
    flat = np.arange(end, dtype=np.float32)
    xp = (bias + (flat % w) * step).astype(np.float32)
    yp = (bias + (flat // w) * step).astype(np.float32)
    freqs = (1.0 / theta ** (np.arange(0, dim, 4, dtype=np.float32)[: dim // 4]
                             / dim)).astype(np.float32)
    xf = np.outer(xp, freqs)
    yf = np.outer(yp, freqs)
    return np.stack([xf, yf], axis=-1).reshape(end, -1).astype(np.float32)


def _host_prep(x, Wq, Wkv, sr_w, sr_b, ln_g, ln_b, proj_w, proj_b):
    f = np.float32
    x = np.asarray(x, f)
    Wq = np.asarray(Wq, f)
    Wkv = np.asarray(Wkv, f)
    sr_w = np.asarray(sr_w, f)
    proj_w = np.asarray(proj_w, f)
    sr_b = np.asarray(sr_b, f)
    ln_g = np.asarray(ln_g, f)
    ln_b = np.asarray(ln_b, f)
    proj_b = np.asarray(proj_b, f)

    # head-dim permutation: pair-first -> rows 0..383, pair-second -> 384..767
    hh = np.arange(HEADS)[:, None] * HD
    jj = np.arange(HD // 2)[None, :] * 2
    perm = np.concatenate([(hh + jj).ravel(), (hh + jj + 1).ravel()])

    Wk = Wkv[:C]
    Wv = Wkv[C:]
    wqT = np.ascontiguousarray(Wq[perm, :].T.astype(BF))
    wkT = np.ascontiguousarray((Wk * ln_g[None, :])[perm, :].T.astype(BF))
    wvT = np.ascontiguousarray((Wv * ln_g[None, :]).T.astype(BF))
    kbias = (Wk @ ln_b)[perm].astype(f)
    vbias = (Wv @ ln_b).astype(f)
    pbias_host = (proj_b + proj_w @ vbias).astype(f)
    srwT = np.ascontiguousarray(sr_w.reshape(C, KC).T.astype(BF))
    projT = np.ascontiguousarray(proj_w.T.astype(BF))

    # im2col of the first 6 image rows, both batches: [3072, 192] pad 256
    strip = x[:, :6 * W, :].reshape(B, 3, 2, 32, 2, C)   # b, i, di, j, dj, c
    xi2c = np.zeros((KC, 256), f)
    xi2c[:, :M2] = strip.transpose(5, 2, 4, 0, 1, 3).reshape(KC, M2)
    xi2c = xi2c.astype(BF)

    # RoPE tables
    ang_q = _angles(HD, N, W)
    ang_k = _angles(HD, N // (SR * SR), W, step=SR, bias=1.0 - 1.0 / SR)
    rowj = np.arange(128) % 32
    cq_full = np.cos(ang_q)[:, rowj].T                   # [128, 4096]
    sq_full = np.sin(ang_q)[:, rowj].T
    ckk = np.cos(ang_k)[:M, rowj].T                      # [128, 96]
    skk = np.sin(ang_k)[:M, rowj].T
    ck2 = np.concatenate([ckk, ckk], 1)
    sk2 = np.concatenate([skk, skk], 1)

    # visibility
    n_all = np.arange(N)
    xpos = n_all // (SR * H)
    ox = n_all // H
    oy = n_all % H
    ypos = (ox + oy * H) // (SR * H)
    vis = xpos * SR + ypos + 1                            # [4096], 1..94

    # mask matrices: A[j, m] = NEG*[m >= j]; B[j, n] = [vis(n) == j]
    A_m = np.zeros((128, M), f)
    jm = np.arange(M)
    A_m[:M, :] = np.where(jm[None, :] >= jm[:, None], NEG, 0.0)

    blob32 = np.zeros((128, 12), f)
    blob32[:, 0:6] = sr_b.reshape(NCH, 128).T
    blob32[:, 6:12] = kbias.reshape(NCH, 128).T

    shared = dict(xi2c=xi2c, wqT=wqT, wkT=wkT, wvT=wvT, srwT=srwT,
                  projT=projT, blob32=blob32)

    in_maps = []
    for core in range(N_CORES):
        ns = slice(core * NQ, (core + 1) * NQ)
        xs = x[:, ns, :]
        xTc = np.ascontiguousarray(
            xs.transpose(2, 0, 1).reshape(C, NQT).astype(BF))
        visc = vis[ns]
        B_m = np.zeros((128, NQ), f)
        B_m[:M, :] = (visc[None, :] == jm[:, None]).astype(f)
        blob = np.zeros((128, BLOB16), f)
        blob[:, O_CQ:O_CQ + NQ] = cq_full[:, ns]
        blob[:, O_CQ + NQ:O_CQ + NQT] = cq_full[:, ns]
        blob[:, O_SQ:O_SQ + NQ] = sq_full[:, ns]
        blob[:, O_SQ + NQ:O_SQ + NQT] = sq_full[:, ns]
        blob[:, O_CK:O_CK + M2] = ck2
        blob[:, O_SK:O_SK + M2] = sk2
        blob[:, O_A:O_A + M] = A_m
        blob[:, O_B:O_B + NQ] = B_m
        in_maps.append(dict(shared, xT=xTc, blob16=blob.astype(BF)))
    return in_maps, pbias_host


_NC_CACHE = {}


def _get_program():
    if "nc" not in _NC_CACHE:
        _NC_CACHE["nc"] = build_program()
    return _NC_CACHE["nc"]


def kernel(x, Wq, Wkv, sr_w, sr_b, ln_g, ln_b, proj_w, proj_b, H=None, W=None,
           _trace=False):
    nc = _get_program()
    in_maps, pbias_host = _host_prep(x, Wq, Wkv, sr_w, sr_b, ln_g, ln_b,
                                     proj_w, proj_b)
    res = run_bass_kernel_spmd(nc, in_maps, list(range(N_CORES)),
                               trace=_trace)
    kernel.last_result = res
    out = np.empty((B, N, C), np.float32)
    for core in range(N_CORES):
        y = np.asarray(res.results[core]["y"]).astype(np.float32)
        out[:, core * NQ:(core + 1) * NQ, :] = y.reshape(B, NQ, C)
    out += pbias_host[None, None, :]
    return out


# revision 23
# speedup vs baseline: 1.8339x; 1.0598x over previous
"""Trainium2 Bass kernel for nn_GSAttention (spatial-reduction attention).

Strategy (v1, bf16)
-------------------
* Queries sharded 512/core over 8 cores; each core handles both batches
  (1024 query rows).  KV path (conv+LN+KV proj) replicated per core but
  only for the 96 reduced tokens the causal mask can ever see.
* All matmul operands are bf16 (1 cyc/row on PE, half the HBM traffic of
  fp32); accumulation stays fp32 in PSUM.  Host pre-packs every operand
  in device layout; all small tables ride in one fused DMA blob.
* LN affine (g, b) is folded into the K/V projection weights on the host;
  the V-side bias collapses into the final projection bias, which is
  added on the host after the gather (it is exact: attention rows sum
  to 1).  K-side bias is applied on-device per-partition during the
  PSUM->SBUF copy.
* The spatial-causal mask is applied as a third matmul accumulated into
  the QK PSUM tile: lhsT = triangular NEG matrix A [96,96], rhs = onehot
  B[j,n] = [vis(n)==j], so z += A[vis(n),m] = NEG*[m>=vis(n)].
* Softmax normalization: V is augmented with a ones column (row 64 of
  the AV output = denominator); 1/den via DVE reciprocal [1,512] -> PE
  ones-matmul broadcast to [64,512] PSUM -> one tensor-mul (DVE/Pool
  alternating) writes the normalized, bf16 attention output.
"""

import os
import sys

for _p in ("/opt/trn_rl_repo", "/root/.axon_site/_ro/trn_rl_repo"):
    if os.path.isdir(_p) and _p not in sys.path:
        sys.path.insert(0, _p)

from contextlib import ExitStack

import numpy as np
import ml_dtypes

import types as _types
if "antenv.axon_hooks" not in sys.modules:
    _axh = _types.ModuleType("antenv.axon_hooks")
    _axh.get_axon_ntff_profile_hook = lambda: None
    sys.modules["antenv.axon_hooks"] = _axh

import concourse.bacc as bacc
import concourse.mybir as mybir
from concourse.tile import TileContext
from concourse.bass_utils import run_bass_kernel_spmd

F32 = mybir.dt.float32
BF16 = mybir.dt.bfloat16
AF = mybir.ActivationFunctionType
ALU = mybir.AluOpType
BF = ml_dtypes.bfloat16

# Problem constants (hardcoded).
N_CORES = 8
B = 2
N = 4096
C = 768
HEADS = 12
HD = 64
SR = 2
H = W = 64
NQ = 512            # queries per core per batch
NQT = B * NQ        # query rows per core
M = 96              # padded visible reduced tokens (real max vis = 94)
M2 = B * M
KC = C * SR * SR    # 3072 conv contraction
SCALE = 1.0 / 8.0
NEG = -60000.0
NCH = C // 128      # 6 feature chunks

# blob16 column offsets
O_CQ = 0
O_SQ = O_CQ + NQT
O_CK = O_SQ + NQT
O_SK = O_CK + M2
O_A = O_SK + M2
O_B = O_A + M
BLOB16 = O_B + NQ


def build_program():
    nc = bacc.Bacc("TRN2", target_bir_lowering=False, debug=False,
                   num_devices=N_CORES)

    def par(name, shape, out=False, dt=F32):
        return nc.declare_dram_parameter(name, list(shape), dt, isOutput=out)

    xT = par("xT", (C, NQT), dt=BF16)
    xi2c = par("xi2c", (KC, 256), dt=BF16)
    wqT = par("wqT", (C, C), dt=BF16)
    wkT = par("wkT", (C, C), dt=BF16)
    wvT = par("wvT", (C, C), dt=BF16)
    srwT = par("srwT", (KC, C), dt=BF16)
    projT = par("projT", (C, C), dt=BF16)
    blob16 = par("blob16", (128, BLOB16), dt=BF16)
    blob32 = par("blob32", (128, 12), dt=F32)
    Y = par("y", (NQT, C), out=True, dt=BF16)

    with TileContext(nc) as tc, ExitStack() as st:
        st.enter_context(nc.allow_low_precision(
            reason="bf16 reciprocal of softmax denominator; rel tol 2e-2"))
        pers = st.enter_context(tc.tile_pool(name="pers", bufs=1))

        # ---- persistent tiles -----------------------------------------
        b16 = pers.tile([128, BLOB16], BF16, tag="b16", name="b16")
        b32 = pers.tile([128, 12], F32, tag="b32", name="b32")
        rotq = [pers.tile([128, NQT], BF16, tag=f"rotq{i}", name=f"rotq{i}")
                for i in range(NCH)]
        rotk = [pers.tile([128, M2], BF16, tag=f"rotk{i}", name=f"rotk{i}")
                for i in range(NCH)]
        vaug = [pers.tile([M, HEADS * (HD + 1)], BF16, tag=f"vaug{b}",
                          name=f"vaug{b}") for b in range(B)]
        attnT = [pers.tile([128, NQT], BF16, tag=f"attnT{i}", name=f"attnT{i}")
                 for i in range(NCH)]
        xln = [pers.tile([128, M2], BF16, tag=f"xln{o}", name=f"xln{o}")
               for o in range(NCH)]
        ones_t = pers.tile([128, 1], BF16, tag="ones", name="ones")
        one1_t = pers.tile([1, 128], BF16, tag="one1", name="one1")
        eps_t = pers.tile([1, 1], F32, tag="eps", name="eps")
        pj_t = pers.tile([128, NCH * C], BF16, tag="pj", name="pj")

        nc.vector.memset(ones_t[:], 1.0)
        nc.vector.memset(one1_t[:], 1.0)
        nc.vector.memset(eps_t[:], 1e-5)

        cq_s = b16[:, O_CQ:O_CQ + NQT]
        sq_s = b16[:, O_SQ:O_SQ + NQT]
        ck_s = b16[:, O_CK:O_CK + M2]
        sk_s = b16[:, O_SK:O_SK + M2]
        A_s = b16[0:M, O_A:O_A + M]
        B_s = b16[0:M, O_B:O_B + NQ]

        with tc.tile_pool(name="pIn", bufs=1) as pIn, \
             tc.tile_pool(name="pSrw", bufs=4) as pSrw, \
             tc.tile_pool(name="pQs", bufs=1) as pQs, \
             tc.tile_pool(name="pT", bufs=2) as pT, \
             tc.tile_pool(name="pS", bufs=2) as pS, \
             tc.tile_pool(name="pD", bufs=3) as pD, \
             tc.tile_pool(name="pBc", bufs=3) as pBc, \
             tc.tile_pool(name="pE", bufs=3) as pE, \
             tc.tile_pool(name="ps", bufs=1, space="PSUM") as ps:

            # ---- fused input DMAs (SP queue) --------------------------
            # Order: conv stream first (it heads the serial KV chain that
            # gates attention), Q-path operands later (Q blocks fill PE
            # stalls and the D(b0) window).
            def big_dma(tile, dram_ap, a):
                nc.sync.dma_start(
                    out=tile[:].rearrange("p (a k) -> p a k", a=a),
                    in_=dram_ap.rearrange("(a p) k -> p a k", p=128))

            nc.sync.dma_start(out=b16[:], in_=blob16[:])
            nc.sync.dma_start(out=b32[:], in_=blob32[:])
            xi_t = pIn.tile([128, 24 * 256], BF16, tag="xi", name="xi")
            big_dma(xi_t, xi2c[:], 24)
            srw_t = []
            for s in range(4):
                t = pSrw.tile([128, NCH * C], BF16, tag="srw", name=f"srw{s}")
                big_dma(t, srwT[s * 768:(s + 1) * 768, :], NCH)
                srw_t.append(t)
            wk_t = pIn.tile([128, NCH * C], BF16, tag="wk", name="wk")
            big_dma(wk_t, wkT[:], NCH)
            wq_t = pIn.tile([128, NCH * C], BF16, tag="wq", name="wq")
            big_dma(wq_t, wqT[:], NCH)
            xT_t = pIn.tile([128, NCH * NQT], BF16, tag="xT", name="xT")
            xT3 = xT_t[:].rearrange("p (a k) -> p a k", a=NCH)
            nc.sync.dma_start(
                out=xT3[:, :, 0:NQ],
                in_=xT[:, 0:NQ].rearrange("(a p) k -> p a k", p=128))
            wv_t = pIn.tile([128, NCH * C], BF16, tag="wv", name="wv")
            big_dma(wv_t, wvT[:], NCH)
            nc.sync.dma_start(
                out=xT3[:, :, NQ:NQT],
                in_=xT[:, NQ:NQT].rearrange("(a p) k -> p a k", p=128))
            big_dma(pj_t, projT[:], NCH)

            # ---- helper defs ------------------------------------------
            q_sb = [pQs.tile([128, NQT], BF16, tag=f"qsb{o}", name=f"qsb{o}")
                    for o in range(NCH)]
            qtag = [0]

            def q_block(o, nh):
                ns = slice(nh * NQ, (nh + 1) * NQ)
                q_ps = ps.tile([128, NQ], F32, tag=f"b{6 + qtag[0] % 2}",
                               name=f"q{o}{nh}")
                qtag[0] += 1
                for cc in range(NCH):
                    nc.tensor.matmul(
                        q_ps[:], wq_t[:, cc * C + o * 128: cc * C + (o + 1) * 128],
                        xT_t[:, cc * NQT + nh * NQ: cc * NQT + (nh + 1) * NQ],
                        start=(cc == 0), stop=(cc == NCH - 1))
                nc.scalar.copy(q_sb[o][:, ns], q_ps[:])

            def rope_half(c, nh):
                hs = slice(nh * NQ, (nh + 1) * NQ)
                t1 = pT.tile([128, NQ], BF16, tag="t1", name="t1")
                t2 = pT.tile([128, NQ], BF16, tag="t2", name="t2")
                nc.vector.tensor_mul(t1[:], q_sb[c][:, hs], cq_s[:, hs])
                nc.vector.tensor_mul(t2[:], q_sb[c + 3][:, hs], sq_s[:, hs])
                nc.vector.tensor_sub(rotq[c][:, hs], t1[:], t2[:])
                t3 = pT.tile([128, NQ], BF16, tag="t3", name="t3")
                t4 = pT.tile([128, NQ], BF16, tag="t4", name="t4")
                nc.vector.tensor_mul(t3[:], q_sb[c + 3][:, hs], cq_s[:, hs])
                nc.vector.tensor_mul(t4[:], q_sb[c][:, hs], sq_s[:, hs])
                nc.vector.tensor_add(rotq[c + 3][:, hs], t3[:], t4[:])

            def e_block(b, nk):
                r0 = b * NQ + nk * 128
                y_sb = pE.tile([128, C], BF16, tag="y", name="y")
                for ph in range(2):
                    y_ps = ps.tile([128, 384], F32, tag=f"b{6 + ph}",
                                   name="yp")
                    for oc in range(NCH):
                        nc.tensor.matmul(
                            y_ps[:], attnT[oc][:, r0:r0 + 128],
                            pj_t[:, oc * C + ph * 384: oc * C + (ph + 1) * 384],
                            start=(oc == 0), stop=(oc == NCH - 1))
                    nc.scalar.copy(y_sb[:, ph * 384:(ph + 1) * 384], y_ps[:])
                nc.sync.dma_start(out=Y[r0:r0 + 128, :], in_=y_sb[:])

            # ---- conv (spatial reduction) -----------------------------
            xr_ps = [ps.tile([128, M2], F32, tag=f"b{o}", name=f"xr{o}")
                     for o in range(NCH)]
            for kc in range(24):
                s, w = kc // 6, kc % 6
                xi_sl = xi_t[:, kc * 256: kc * 256 + M2]
                for o in range(NCH):
                    nc.tensor.matmul(
                        xr_ps[o][:],
                        srw_t[s][:, w * C + o * 128: w * C + (o + 1) * 128],
                        xi_sl, start=(kc == 0), stop=(kc == 23))

            # ---- LN (stats via ones-matmul; affine folded on host) ----
            xr_sb = [pS.tile([128, M2], BF16, tag=f"xrs{o}", name=f"xrs{o}")
                     for o in range(NCH)]
            for o in range(NCH):
                nc.scalar.activation(xr_sb[o][:], xr_ps[o][:], AF.Identity,
                                     bias=b32[:, o:o + 1])
            sum_ps = ps.tile([1, M2], F32, tag="b0", name="sum")
            for o in range(NCH):
                nc.tensor.matmul(sum_ps[:], ones_t[:], xr_sb[o][:],
                                 start=(o == 0), stop=(o == NCH - 1))
            ssq_ps = ps.tile([1, M2], F32, tag="b1", name="ssq")
            for o in range(NCH):
                sqt = pS.tile([128, M2], BF16, tag="sqt", name="sqt")
                nc.vector.tensor_mul(sqt[:], xr_sb[o][:], xr_sb[o][:])
                nc.tensor.matmul(ssq_ps[:], ones_t[:], sqt[:],
                                 start=(o == 0), stop=(o == NCH - 1))
            mu = pS.tile([1, M2], F32, tag="mu", name="mu")
            mu2 = pS.tile([1, M2], F32, tag="mu2", name="mu2")
            var = pS.tile([1, M2], F32, tag="var", name="var")
            std = pS.tile([1, M2], F32, tag="std", name="std")
            bsrc = pS.tile([1, 2 * M2], BF16, tag="bsrc", name="bsrc")
            nc.scalar.mul(mu[:], sum_ps[:], 1.0 / C)
            nc.vector.tensor_mul(mu2[:], mu[:], mu[:])
            nc.vector.scalar_tensor_tensor(var[:], ssq_ps[:], 1.0 / C, mu2[:],
                                           ALU.mult, ALU.subtract)
            nc.scalar.activation(std[:], var[:], AF.Sqrt, bias=eps_t[:])
            nc.vector.reciprocal(bsrc[:, 0:M2], std[:])
            nc.vector.scalar_tensor_tensor(bsrc[:, M2:2 * M2], mu[:], -1.0,
                                           bsrc[:, 0:M2], ALU.mult, ALU.mult)
            bc_ps = ps.tile([128, 2 * M2], F32, tag="b2", name="bc")
            nc.tensor.matmul(bc_ps[:], one1_t[:], bsrc[:], start=True,
                             stop=True)
            bc_sb = pS.tile([128, 2 * M2], BF16, tag="bcs", name="bcs")
            nc.scalar.copy(bc_sb[:], bc_ps[:])
            for o in range(NCH):
                t = pS.tile([128, M2], BF16, tag="lnt", name="lnt")
                nc.vector.tensor_mul(t[:], xr_sb[o][:], bc_sb[:, 0:M2])
                nc.vector.tensor_add(xln[o][:], t[:], bc_sb[:, M2:2 * M2])

            # ---- K projection (+bias) + RoPE --------------------------
            k_ps = [ps.tile([128, M2], F32, tag=f"b{o}", name=f"k{o}")
                    for o in range(NCH)]
            for o in range(NCH):
                for cc in range(NCH):
                    nc.tensor.matmul(
                        k_ps[o][:], wk_t[:, cc * C + o * 128: cc * C + (o + 1) * 128],
                        xln[cc][:], start=(cc == 0), stop=(cc == NCH - 1))
            k_sb = [pS.tile([128, M2], BF16, tag=f"ksb{o}", name=f"ksb{o}")
                    for o in range(NCH)]
            for o in range(NCH):
                nc.scalar.activation(k_sb[o][:], k_ps[o][:], AF.Identity,
                                     bias=b32[:, 6 + o:7 + o])
            for c in range(3):
                t1 = pS.tile([128, M2], BF16, tag="kt1", name="kt1")
                t2 = pS.tile([128, M2], BF16, tag="kt2", name="kt2")
                nc.vector.tensor_mul(t1[:], k_sb[c][:], ck_s)
                nc.vector.tensor_mul(t2[:], k_sb[c + 3][:], sk_s)
                nc.vector.tensor_sub(rotk[c][:], t1[:], t2[:])
                t3 = pS.tile([128, M2], BF16, tag="kt3", name="kt3")
                t4 = pS.tile([128, M2], BF16, tag="kt4", name="kt4")
                nc.vector.tensor_mul(t3[:], k_sb[c + 3][:], ck_s)
                nc.vector.tensor_mul(t4[:], k_sb[c][:], sk_s)
                nc.vector.tensor_add(rotk[c + 3][:], t3[:], t4[:])

            # ---- first Q pair (fills the PE stall before wv arrives) --
            q_block(0, 0)
            q_block(3, 0)
            rope_half(0, 0)

            # ---- V projection into 65-col augmented layout ------------
            for b in range(B):
                for half in range(2):
                    v_ps = ps.tile([M, 384], F32, tag=f"b{2 * b + half}",
                                   name=f"v{b}{half}")
                    for cc in range(NCH):
                        nc.tensor.matmul(
                            v_ps[:], xln[cc][:, b * M:(b + 1) * M],
                            wv_t[:, cc * C + half * 384: cc * C + (half + 1) * 384],
                            start=(cc == 0), stop=(cc == NCH - 1))
                    dst = vaug[b][:].rearrange("p (h d) -> p h d", d=HD + 1)
                    src_ = v_ps[:].rearrange("p (h d) -> p h d", d=HD)
                    nc.vector.tensor_copy(
                        dst[:, half * 6:(half + 1) * 6, 0:HD], src_)
                ocol = vaug[b][:].rearrange("p (h d) -> p h d", d=HD + 1)
                nc.vector.memset(ocol[:, :, HD:HD + 1], 1.0)

            # ================= Phase D + E =============================
            # Per (batch, head) unit: z = k^T q + mask (3 accumulated
            # matmuls), e = exp(z/8) [Act], u = V_aug^T e [PE] (row 64 =
            # denominator), rec = 1/den [DVE], partition-broadcast [Pool],
            # one normalize-mul [DVE] -> bf16 attnT.  Software-pipelined so
            # the PE never waits; remaining Q blocks / RoPE halves / E(b0)
            # blocks are interleaved as PE fillers.
            units = [(b, h) for b in range(B) for h in range(HEADS)]
            fillers = {
                0: [("q", 1, 0), ("q", 4, 0)], 1: [("r", 1, 0)],
                2: [("q", 2, 0), ("q", 5, 0)], 3: [("r", 2, 0)],
                4: [("q", 0, 1), ("q", 3, 1)], 5: [("r", 0, 1)],
                6: [("q", 1, 1), ("q", 4, 1)], 7: [("r", 1, 1)],
                8: [("q", 2, 1), ("q", 5, 1)], 9: [("r", 2, 1)],
                15: [("e", 0, 0)], 18: [("e", 0, 1)],
                21: [("e", 0, 2)], 24: [("e", 0, 3)],
            }
            es, us, bcs = {}, {}, {}
            for i in range(len(units) + 3):
                if i < len(units):
                    b, h = units[i]
                    hq, hr = h // 4, (h % 4) * 32
                    rs = slice(hr, hr + 32)
                    ms = slice(b * M, (b + 1) * M)
                    qs = slice(b * NQ, (b + 1) * NQ)
                    z_ps = ps.tile([M, NQ], F32, tag=f"b{4 + i % 2}",
                                   name="z")
                    tp = (hr, 0) if hr == 96 else None
                    nc.tensor.matmul(z_ps[:], rotk[hq][rs, ms],
                                     rotq[hq][rs, qs], start=True,
                                     stop=False, tile_position=tp)
                    nc.tensor.matmul(z_ps[:], rotk[hq + 3][rs, ms],
                                     rotq[hq + 3][rs, qs], start=False,
                                     stop=False, tile_position=tp)
                    nc.tensor.matmul(z_ps[:], A_s, B_s, start=False,
                                     stop=True)
                    e_sb = pD.tile([M, NQ], BF16, tag=f"e{i % 2}", name="e")
                    nc.scalar.activation(e_sb[:], z_ps[:], AF.Exp,
                                         scale=SCALE)
                    es[i] = e_sb
                if i >= 1 and i - 1 < len(units):
                    j = i - 1
                    b, h = units[j]
                    u_ps = ps.tile([HD + 1, NQ], F32, tag=f"b{j % 4}",
                                   name="u")
                    nc.tensor.matmul(
                        u_ps[:],
                        vaug[b][:, h * (HD + 1):(h + 1) * (HD + 1)],
                        es.pop(j)[:], start=True, stop=True)
                    rec = pD.tile([1, NQ], BF16, tag="rec", name="rec")
                    nc.vector.reciprocal(rec[:], u_ps[HD:HD + 1, :])
                    bc = pBc.tile([HD, NQ], BF16, tag="bc", name="bc")
                    nc.gpsimd.partition_broadcast(bc[:], rec[:])
                    us[j], bcs[j] = u_ps, bc
                if i >= 3:
                    j = i - 3
                    b, h = units[j]
                    dst = attnT[h // 2][(h % 2) * HD:(h % 2 + 1) * HD,
                                        b * NQ:(b + 1) * NQ]
                    nc.vector.tensor_mul(dst, us[j][0:HD, :], bcs[j][:])
                    del us[j], bcs[j]
                for f in fillers.get(i, ()):
                    if f[0] == "q":
                        q_block(f[1], f[2])
                    elif f[0] == "r":
                        rope_half(f[1], f[2])
                    else:
                        e_block(f[1], f[2])
            for nk in range(4):
                e_block(1, nk)

    nc.compile()
    return nc


# ======================= host-side preparation =======================

def _angles(dim, end, w, step=1.0, bias=0.0, theta=10000.0):
    flat = np.arange(end, dtype=np.float32)
    xp = (bias + (flat % w) * step).astype(np.float32)
    yp = (bias + (flat // w) * step).astype(np.float32)
    freqs = (1.0 / theta ** (np.arange(0, dim, 4, dtype=np.float32)[: dim // 4]
                             / dim)).astype(np.float32)
    xf = np.outer(xp, freqs)
    yf = np.outer(yp, freqs)
    return np.stack([xf, yf], axis=-1).reshape(end, -1).astype(np.float32)


def _host_prep(x, Wq, Wkv, sr_w, sr_b, ln_g, ln_b, proj_w, proj_b):
    f = np.float32
    x = np.asarray(x, f)
    Wq = np.asarray(Wq, f)
    Wkv = np.asarray(Wkv, f)
    sr_w = np.asarray(sr_w, f)
    proj_w = np.asarray(proj_w, f)
    sr_b = np.asarray(sr_b, f)
    ln_g = np.asarray(ln_g, f)
    ln_b = np.asarray(ln_b, f)
    proj_b = np.asarray(proj_b, f)

    # head-dim permutation: pair-first -> rows 0..383, pair-second -> 384..767
    hh = np.arange(HEADS)[:, None] * HD
    jj = np.arange(HD // 2)[None, :] * 2
    perm = np.concatenate([(hh + jj).ravel(), (hh + jj + 1).ravel()])

    Wk = Wkv[:C]
    Wv = Wkv[C:]
    wqT = np.ascontiguousarray(Wq[perm, :].T.astype(BF))
    wkT = np.ascontiguousarray((Wk * ln_g[None, :])[perm, :].T.astype(BF))
    wvT = np.ascontiguousarray((Wv * ln_g[None, :]).T.astype(BF))
    kbias = (Wk @ ln_b)[perm].astype(f)
    vbias = (Wv @ ln_b).astype(f)
    pbias_host = (proj_b + proj_w @ vbias).astype(f)
    srwT = np.ascontiguousarray(sr_w.reshape(C, KC).T.astype(BF))
    projT = np.ascontiguousarray(proj_w.T.astype(BF))

    # im2col of the first 6 image rows, both batches: [3072, 192] pad 256
    strip = x[:, :6 * W, :].reshape(B, 3, 2, 32, 2, C)   # b, i, di, j, dj, c
    xi2c = np.zeros((KC, 256), f)
    xi2c[:, :M2] = strip.transpose(5, 2, 4, 0, 1, 3).reshape(KC, M2)
    xi2c = xi2c.astype(BF)

    # RoPE tables
    ang_q = _angles(HD, N, W)
    ang_k = _angles(HD, N // (SR * SR), W, step=SR, bias=1.0 - 1.0 / SR)
    rowj = np.arange(128) % 32
    cq_full = np.cos(ang_q)[:, rowj].T                   # [128, 4096]
    sq_full = np.sin(ang_q)[:, rowj].T
    ckk = np.cos(ang_k)[:M, rowj].T                      # [128, 96]
    skk = np.sin(ang_k)[:M, rowj].T
    ck2 = np.concatenate([ckk, ckk], 1)
    sk2 = np.concatenate([skk, skk], 1)

    # visibility
    n_all = np.arange(N)
    xpos = n_all // (SR * H)
    ox = n_all // H
    oy = n_all % H
    ypos = (ox + oy * H) // (SR * H)
    vis = xpos * SR + ypos + 1                            # [4096], 1..94

    # mask matrices: A[j, m] = NEG*[m >= j]; B[j, n] = [vis(n) == j]
    A_m = np.zeros((128, M), f)
    jm = np.arange(M)
    A_m[:M, :] = np.where(jm[None, :] >= jm[:, None], NEG, 0.0)

    blob32 = np.zeros((128, 12), f)
    blob32[:, 0:6] = sr_b.reshape(NCH, 128).T
    blob32[:, 6:12] = kbias.reshape(NCH, 128).T

    shared = dict(xi2c=xi2c, wqT=wqT, wkT=wkT, wvT=wvT, srwT=srwT,
                  projT=projT, blob32=blob32)

    in_maps = []
    for core in range(N_CORES):
        ns = slice(core * NQ, (core + 1) * NQ)
        xs = x[:, ns, :]
        xTc = np.ascontiguousarray(
            xs.transpose(2, 0, 1).reshape(C, NQT).astype(BF))
        visc = vis[ns]
        B_m = np.zeros((128, NQ), f)
        B_m[:M, :] = (visc[None, :] == jm[:, None]).astype(f)
        blob = np.zeros((128, BLOB16), f)
        blob[:, O_CQ:O_CQ + NQ] = cq_full[:, ns]
        blob[:, O_CQ + NQ:O_CQ + NQT] = cq_full[:, ns]
        blob[:, O_SQ:O_SQ + NQ] = sq_full[:, ns]
        blob[:, O_SQ + NQ:O_SQ + NQT] = sq_full[:, ns]
        blob[:, O_CK:O_CK + M2] = ck2
        blob[:, O_SK:O_SK + M2] = sk2
        blob[:, O_A:O_A + M] = A_m
        blob[:, O_B:O_B + NQ] = B_m
        in_maps.append(dict(shared, xT=xTc, blob16=blob.astype(BF)))
    return in_maps, pbias_host


_NC_CACHE = {}


def _get_program():
    if "nc" not in _NC_CACHE:
        _NC_CACHE["nc"] = build_program()
    return _NC_CACHE["nc"]


def kernel(x, Wq, Wkv, sr_w, sr_b, ln_g, ln_b, proj_w, proj_b, H=None, W=None,
           _trace=False):
    nc = _get_program()
    in_maps, pbias_host = _host_prep(x, Wq, Wkv, sr_w, sr_b, ln_g, ln_b,
                                     proj_w, proj_b)
    res = run_bass_kernel_spmd(nc, in_maps, list(range(N_CORES)),
                               trace=_trace)
    kernel.last_result = res
    out = np.empty((B, N, C), np.float32)
    for core in range(N_CORES):
        y = np.asarray(res.results[core]["y"]).astype(np.float32)
        out[:, core * NQ:(core + 1) * NQ, :] = y.reshape(B, NQ, C)
    out += pbias_host[None, None, :]
    return out


# revision 27
# speedup vs baseline: 1.8581x; 1.0132x over previous
"""Trainium2 Bass kernel for nn_GSAttention (spatial-reduction attention).

Strategy (v1, bf16)
-------------------
* Queries sharded 512/core over 8 cores; each core handles both batches
  (1024 query rows).  KV path (conv+LN+KV proj) replicated per core but
  only for the 96 reduced tokens the causal mask can ever see.
* All matmul operands are bf16 (1 cyc/row on PE, half the HBM traffic of
  fp32); accumulation stays fp32 in PSUM.  Host pre-packs every operand
  in device layout; all small tables ride in one fused DMA blob.
* LN affine (g, b) is folded into the K/V projection weights on the host;
  the V-side bias collapses into the final projection bias, which is
  added on the host after the gather (it is exact: attention rows sum
  to 1).  K-side bias is applied on-device per-partition during the
  PSUM->SBUF copy.
* The spatial-causal mask is applied as a third matmul accumulated into
  the QK PSUM tile: lhsT = triangular NEG matrix A [96,96], rhs = onehot
  B[j,n] = [vis(n)==j], so z += A[vis(n),m] = NEG*[m>=vis(n)].
* Softmax normalization: V is augmented with a ones column (row 64 of
  the AV output = denominator); 1/den via DVE reciprocal [1,512] -> PE
  ones-matmul broadcast to [64,512] PSUM -> one tensor-mul (DVE/Pool
  alternating) writes the normalized, bf16 attention output.
"""

import os
import sys

for _p in ("/opt/trn_rl_repo", "/root/.axon_site/_ro/trn_rl_repo"):
    if os.path.isdir(_p) and _p not in sys.path:
        sys.path.insert(0, _p)

from contextlib import ExitStack

import numpy as np
import ml_dtypes

import types as _types
if "antenv.axon_hooks" not in sys.modules:
    _axh = _types.ModuleType("antenv.axon_hooks")
    _axh.get_axon_ntff_profile_hook = lambda: None
    sys.modules["antenv.axon_hooks"] = _axh

import concourse.bacc as bacc
import concourse.mybir as mybir
from concourse.tile import TileContext
from concourse.bass_utils import run_bass_kernel_spmd

F32 = mybir.dt.float32
BF16 = mybir.dt.bfloat16
FP8 = mybir.dt.float8e4
AF = mybir.ActivationFunctionType
ALU = mybir.AluOpType
BF = ml_dtypes.bfloat16
F8 = ml_dtypes.float8_e4m3fn
CONV_WS = 8.0       # host pre-scale on conv weights (undone in xr copy)

# Problem constants (hardcoded).
N_CORES = 8
B = 2
N = 4096
C = 768
HEADS = 12
HD = 64
SR = 2
H = W = 64
NQ = 512            # queries per core per batch
NQT = B * NQ        # query rows per core
M = 96              # padded visible reduced tokens (real max vis = 94)
M2 = B * M
KC = C * SR * SR    # 3072 conv contraction
SCALE = 1.0 / 8.0
NEG = -60000.0
NCH = C // 128      # 6 feature chunks

# blob16 column offsets
O_CQ = 0
O_SQ = O_CQ + NQT
O_CK = O_SQ + NQT
O_SK = O_CK + M2
O_A = O_SK + M2
O_B = O_A + M
BLOB16 = O_B + NQ


def build_program():
    nc = bacc.Bacc("TRN2", target_bir_lowering=False, debug=False,
                   num_devices=N_CORES)

    def par(name, shape, out=False, dt=F32):
        return nc.declare_dram_parameter(name, list(shape), dt, isOutput=out)

    xT = par("xT", (C, NQT), dt=BF16)
    xi2c = par("xi2c", (KC, 256), dt=BF16)
    wqT = par("wqT", (C, C), dt=BF16)
    wkT = par("wkT", (C, C), dt=BF16)
    wvT = par("wvT", (C, C), dt=BF16)
    srwT = par("srwT", (KC, C), dt=BF16)
    projT = par("projT", (C, C), dt=BF16)
    blob16 = par("blob16", (128, BLOB16), dt=BF16)
    blob32 = par("blob32", (128, 12), dt=F32)
    Y = par("y", (NQT, C), out=True, dt=BF16)

    with TileContext(nc) as tc, ExitStack() as st:
        st.enter_context(nc.allow_low_precision(
            reason="bf16 reciprocal of softmax denominator; rel tol 2e-2"))
        pers = st.enter_context(tc.tile_pool(name="pers", bufs=1))

        # ---- persistent tiles -----------------------------------------
        b16 = pers.tile([128, BLOB16], BF16, tag="b16", name="b16")
        b32 = pers.tile([128, 12], F32, tag="b32", name="b32")
        rotq = [pers.tile([128, NQT], BF16, tag=f"rotq{i}", name=f"rotq{i}")
                for i in range(NCH)]
        rotk = [pers.tile([128, M2], BF16, tag=f"rotk{i}", name=f"rotk{i}")
                for i in range(NCH)]
        vaug = [pers.tile([M, HEADS * (HD + 1)], BF16, tag=f"vaug{b}",
                          name=f"vaug{b}") for b in range(B)]
        attnT = [pers.tile([128, NQT], BF16, tag=f"attnT{i}", name=f"attnT{i}")
                 for i in range(NCH)]
        xln = [pers.tile([128, M2], BF16, tag=f"xln{o}", name=f"xln{o}")
               for o in range(NCH)]
        ones_t = pers.tile([128, 1], BF16, tag="ones", name="ones")
        one1_t = pers.tile([1, 128], BF16, tag="one1", name="one1")
        eps_t = pers.tile([1, 1], F32, tag="eps", name="eps")
        pj_t = pers.tile([128, NCH * C], BF16, tag="pj", name="pj")

        nc.vector.memset(ones_t[:], 1.0)
        nc.vector.memset(one1_t[:], 1.0)
        nc.vector.memset(eps_t[:], 1e-5)

        cq_s = b16[:, O_CQ:O_CQ + NQT]
        sq_s = b16[:, O_SQ:O_SQ + NQT]
        ck_s = b16[:, O_CK:O_CK + M2]
        sk_s = b16[:, O_SK:O_SK + M2]
        A_s = b16[0:M, O_A:O_A + M]
        B_s = b16[0:M, O_B:O_B + NQ]

        with tc.tile_pool(name="pIn", bufs=1) as pIn, \
             tc.tile_pool(name="pSrw", bufs=4) as pSrw, \
             tc.tile_pool(name="pQs", bufs=1) as pQs, \
             tc.tile_pool(name="pT", bufs=2) as pT, \
             tc.tile_pool(name="pS", bufs=2) as pS, \
             tc.tile_pool(name="pD", bufs=3) as pD, \
             tc.tile_pool(name="pBc", bufs=3) as pBc, \
             tc.tile_pool(name="pE", bufs=3) as pE, \
             tc.tile_pool(name="ps", bufs=1, space="PSUM") as ps:

            # ---- fused input DMAs (SP queue) --------------------------
            # Order: conv stream first (it heads the serial KV chain that
            # gates attention), Q-path operands later (Q blocks fill PE
            # stalls and the D(b0) window).
            def big_dma(tile, dram_ap, a):
                nc.sync.dma_start(
                    out=tile[:].rearrange("p (a k) -> p a k", a=a),
                    in_=dram_ap.rearrange("(a p) k -> p a k", p=128))

            nc.sync.dma_start(out=b16[:], in_=blob16[:])
            nc.sync.dma_start(out=b32[:], in_=blob32[:])
            xi_t = pIn.tile([128, 24 * 256], BF16, tag="xi", name="xi")
            big_dma(xi_t, xi2c[:], 24)
            srw_t = []
            for s in range(4):
                t = pSrw.tile([128, NCH * C], BF16, tag="srw", name=f"srw{s}")
                big_dma(t, srwT[s * 768:(s + 1) * 768, :], NCH)
                srw_t.append(t)
            wk_t = pIn.tile([128, NCH * C], BF16, tag="wk", name="wk")
            big_dma(wk_t, wkT[:], NCH)
            wq_t = pIn.tile([128, NCH * C], BF16, tag="wq", name="wq")
            big_dma(wq_t, wqT[:], NCH)
            xT_t = pIn.tile([128, NCH * NQT], BF16, tag="xT", name="xT")
            xT3 = xT_t[:].rearrange("p (a k) -> p a k", a=NCH)
            nc.sync.dma_start(
                out=xT3[:, :, 0:NQ],
                in_=xT[:, 0:NQ].rearrange("(a p) k -> p a k", p=128))
            wv_t = pIn.tile([128, NCH * C], BF16, tag="wv", name="wv")
            big_dma(wv_t, wvT[:], NCH)
            nc.sync.dma_start(
                out=xT3[:, :, NQ:NQT],
                in_=xT[:, NQ:NQT].rearrange("(a p) k -> p a k", p=128))
            big_dma(pj_t, projT[:], NCH)

            # ---- helper defs ------------------------------------------
            q_sb = [pQs.tile([128, NQT], BF16, tag=f"qsb{o}", name=f"qsb{o}")
                    for o in range(NCH)]
            qtag = [0]

            def q_block(o, nh):
                ns = slice(nh * NQ, (nh + 1) * NQ)
                q_ps = ps.tile([128, NQ], F32, tag=f"b{6 + qtag[0] % 2}",
                               name=f"q{o}{nh}")
                qtag[0] += 1
                for cc in range(NCH):
                    nc.tensor.matmul(
                        q_ps[:], wq_t[:, cc * C + o * 128: cc * C + (o + 1) * 128],
                        xT_t[:, cc * NQT + nh * NQ: cc * NQT + (nh + 1) * NQ],
                        start=(cc == 0), stop=(cc == NCH - 1))
                nc.scalar.copy(q_sb[o][:, ns], q_ps[:])

            def rope_half(c, nh, pool=False):
                hs = slice(nh * NQ, (nh + 1) * NQ)
                mul1 = nc.gpsimd.tensor_mul if pool else nc.vector.tensor_mul
                t1 = pT.tile([128, NQ], BF16, tag="t1", name="t1")
                t2 = pT.tile([128, NQ], BF16, tag="t2", name="t2")
                mul1(t1[:], q_sb[c][:, hs], cq_s[:, hs])
                nc.vector.tensor_mul(t2[:], q_sb[c + 3][:, hs], sq_s[:, hs])
                nc.vector.tensor_sub(rotq[c][:, hs], t1[:], t2[:])
                t3 = pT.tile([128, NQ], BF16, tag="t3", name="t3")
                t4 = pT.tile([128, NQ], BF16, tag="t4", name="t4")
                mul1(t3[:], q_sb[c + 3][:, hs], cq_s[:, hs])
                nc.vector.tensor_mul(t4[:], q_sb[c][:, hs], sq_s[:, hs])
                nc.vector.tensor_add(rotq[c + 3][:, hs], t3[:], t4[:])

            def e_block(b, nk):
                r0 = b * NQ + nk * 128
                y_sb = pE.tile([128, C], BF16, tag="y", name="y")
                for ph in range(2):
                    y_ps = ps.tile([128, 384], F32, tag=f"b{6 + ph}",
                                   name="yp")
                    for oc in range(NCH):
                        nc.tensor.matmul(
                            y_ps[:], attnT[oc][:, r0:r0 + 128],
                            pj_t[:, oc * C + ph * 384: oc * C + (ph + 1) * 384],
                            start=(oc == 0), stop=(oc == NCH - 1))
                    nc.scalar.copy(y_sb[:, ph * 384:(ph + 1) * 384], y_ps[:])
                nc.sync.dma_start(out=Y[r0:r0 + 128, :], in_=y_sb[:])

            # ---- conv (spatial reduction) -----------------------------
            xr_ps = [ps.tile([128, M2], F32, tag=f"b{o}", name=f"xr{o}")
                     for o in range(NCH)]
            for kc in range(24):
                s, w = kc // 6, kc % 6
                xi_sl = xi_t[:, kc * 256: kc * 256 + M2]
                for o in range(NCH):
                    nc.tensor.matmul(
                        xr_ps[o][:],
                        srw_t[s][:, w * C + o * 128: w * C + (o + 1) * 128],
                        xi_sl, start=(kc == 0), stop=(kc == 23))

            # ---- LN (stats via ones-matmul; affine folded on host) ----
            xr_sb = [pS.tile([128, M2], BF16, tag=f"xrs{o}", name=f"xrs{o}")
                     for o in range(NCH)]
            for o in range(NCH):
                nc.scalar.activation(xr_sb[o][:], xr_ps[o][:], AF.Identity,
                                     bias=b32[:, o:o + 1])
            sum_ps = ps.tile([1, M2], F32, tag="b0", name="sum")
            for o in range(NCH):
                nc.tensor.matmul(sum_ps[:], ones_t[:], xr_sb[o][:],
                                 start=(o == 0), stop=(o == NCH - 1))
            ssq_ps = ps.tile([1, M2], F32, tag="b1", name="ssq")
            for o in range(NCH):
                sqt = pS.tile([128, M2], BF16, tag="sqt", name="sqt")
                nc.vector.tensor_mul(sqt[:], xr_sb[o][:], xr_sb[o][:])
                nc.tensor.matmul(ssq_ps[:], ones_t[:], sqt[:],
                                 start=(o == 0), stop=(o == NCH - 1))
            mu = pS.tile([1, M2], F32, tag="mu", name="mu")
            mu2 = pS.tile([1, M2], F32, tag="mu2", name="mu2")
            var = pS.tile([1, M2], F32, tag="var", name="var")
            std = pS.tile([1, M2], F32, tag="std", name="std")
            bsrc = pS.tile([1, 2 * M2], BF16, tag="bsrc", name="bsrc")
            nc.scalar.mul(mu[:], sum_ps[:], 1.0 / C)
            nc.vector.tensor_mul(mu2[:], mu[:], mu[:])
            nc.vector.scalar_tensor_tensor(var[:], ssq_ps[:], 1.0 / C, mu2[:],
                                           ALU.mult, ALU.subtract)
            nc.scalar.activation(std[:], var[:], AF.Sqrt, bias=eps_t[:])
            nc.vector.reciprocal(bsrc[:, 0:M2], std[:])
            nc.vector.scalar_tensor_tensor(bsrc[:, M2:2 * M2], mu[:], -1.0,
                                           bsrc[:, 0:M2], ALU.mult, ALU.mult)
            bc_ps = ps.tile([128, 2 * M2], F32, tag="b2", name="bc")
            nc.tensor.matmul(bc_ps[:], one1_t[:], bsrc[:], start=True,
                             stop=True)
            bc_sb = pS.tile([128, 2 * M2], BF16, tag="bcs", name="bcs")
            nc.scalar.copy(bc_sb[:], bc_ps[:])
            for o in range(NCH):
                t = pS.tile([128, M2], BF16, tag="lnt", name="lnt")
                nc.vector.tensor_mul(t[:], xr_sb[o][:], bc_sb[:, 0:M2])
                nc.vector.tensor_add(xln[o][:], t[:], bc_sb[:, M2:2 * M2])

            # ---- K projection (+bias) + RoPE --------------------------
            k_ps = [ps.tile([128, M2], F32, tag=f"b{o}", name=f"k{o}")
                    for o in range(NCH)]
            for o in range(NCH):
                for cc in range(NCH):
                    nc.tensor.matmul(
                        k_ps[o][:], wk_t[:, cc * C + o * 128: cc * C + (o + 1) * 128],
                        xln[cc][:], start=(cc == 0), stop=(cc == NCH - 1))
            k_sb = [pS.tile([128, M2], BF16, tag=f"ksb{o}", name=f"ksb{o}")
                    for o in range(NCH)]
            for o in range(NCH):
                nc.scalar.activation(k_sb[o][:], k_ps[o][:], AF.Identity,
                                     bias=b32[:, 6 + o:7 + o])
            for c in range(3):
                t1 = pS.tile([128, M2], BF16, tag="kt1", name="kt1")
                t2 = pS.tile([128, M2], BF16, tag="kt2", name="kt2")
                nc.vector.tensor_mul(t1[:], k_sb[c][:], ck_s)
                nc.vector.tensor_mul(t2[:], k_sb[c + 3][:], sk_s)
                nc.vector.tensor_sub(rotk[c][:], t1[:], t2[:])
                t3 = pS.tile([128, M2], BF16, tag="kt3", name="kt3")
                t4 = pS.tile([128, M2], BF16, tag="kt4", name="kt4")
                nc.vector.tensor_mul(t3[:], k_sb[c + 3][:], ck_s)
                nc.vector.tensor_mul(t4[:], k_sb[c][:], sk_s)
                nc.vector.tensor_add(rotk[c + 3][:], t3[:], t4[:])

            # ---- first Q pair (fills the PE stall before wv arrives) --
            q_block(0, 0)
            q_block(3, 0)
            rope_half(0, 0)

            # ---- V projection into 65-col augmented layout ------------
            for b in range(B):
                for half in range(2):
                    v_ps = ps.tile([M, 384], F32, tag=f"b{2 * b + half}",
                                   name=f"v{b}{half}")
                    for cc in range(NCH):
                        nc.tensor.matmul(
                            v_ps[:], xln[cc][:, b * M:(b + 1) * M],
                            wv_t[:, cc * C + half * 384: cc * C + (half + 1) * 384],
                            start=(cc == 0), stop=(cc == NCH - 1))
                    dst = vaug[b][:].rearrange("p (h d) -> p h d", d=HD + 1)
                    src_ = v_ps[:].rearrange("p (h d) -> p h d", d=HD)
                    nc.vector.tensor_copy(
                        dst[:, half * 6:(half + 1) * 6, 0:HD], src_)
                ocol = vaug[b][:].rearrange("p (h d) -> p h d", d=HD + 1)
                nc.vector.memset(ocol[:, :, HD:HD + 1], 1.0)

            # ================= Phase D + E =============================
            # Per (batch, head) unit: z = k^T q + mask (3 accumulated
            # matmuls), e = exp(z/8) [Act], u = V_aug^T e [PE] (row 64 =
            # denominator), rec = 1/den [DVE], partition-broadcast [Pool],
            # one normalize-mul [DVE] -> bf16 attnT.  Software-pipelined so
            # the PE never waits; remaining Q blocks / RoPE halves / E(b0)
            # blocks are interleaved as PE fillers.
            units = [(b, h) for b in range(B) for h in range(HEADS)]
            fillers = {
                0: [("q", 1, 0), ("q", 4, 0)], 1: [("r", 1, 0)],
                2: [("q", 2, 0), ("q", 5, 0)], 3: [("r", 2, 0)],
                4: [("q", 0, 1), ("q", 3, 1)], 5: [("r", 0, 1)],
                6: [("q", 1, 1), ("q", 4, 1)], 7: [("r", 1, 1)],
                8: [("q", 2, 1), ("q", 5, 1)], 9: [("r", 2, 1)],
                15: [("e", 0, 0)], 18: [("e", 0, 1)],
                21: [("e", 0, 2)], 24: [("e", 0, 3)],
            }
            es, us, bcs = {}, {}, {}
            for i in range(len(units) + 3):
                if i < len(units):
                    b, h = units[i]
                    hq, hr = h // 4, (h % 4) * 32
                    rs = slice(hr, hr + 32)
                    ms = slice(b * M, (b + 1) * M)
                    qs = slice(b * NQ, (b + 1) * NQ)
                    z_ps = ps.tile([M, NQ], F32, tag=f"b{4 + i % 2}",
                                   name="z")
                    tp = (hr, 0) if hr == 96 else None
                    nc.tensor.matmul(z_ps[:], rotk[hq][rs, ms],
                                     rotq[hq][rs, qs], start=True,
                                     stop=False, tile_position=tp)
                    nc.tensor.matmul(z_ps[:], rotk[hq + 3][rs, ms],
                                     rotq[hq + 3][rs, qs], start=False,
                                     stop=False, tile_position=tp)
                    nc.tensor.matmul(z_ps[:], A_s, B_s, start=False,
                                     stop=True)
                    e_sb = pD.tile([M, NQ], BF16, tag=f"e{i % 2}", name="e")
                    nc.scalar.activation(e_sb[:], z_ps[:], AF.Exp,
                                         scale=SCALE)
                    es[i] = e_sb
                if i >= 1 and i - 1 < len(units):
                    j = i - 1
                    b, h = units[j]
                    u_ps = ps.tile([HD + 1, NQ], F32, tag=f"b{j % 4}",
                                   name="u")
                    nc.tensor.matmul(
                        u_ps[:],
                        vaug[b][:, h * (HD + 1):(h + 1) * (HD + 1)],
                        es.pop(j)[:], start=True, stop=True)
                    rec = pD.tile([1, NQ], BF16, tag="rec", name="rec")
                    nc.vector.reciprocal(rec[:], u_ps[HD:HD + 1, :])
                    bc = pBc.tile([HD, NQ], BF16, tag="bc", name="bc")
                    nc.gpsimd.partition_broadcast(bc[:], rec[:])
                    us[j], bcs[j] = u_ps, bc
                if i >= 3:
                    j = i - 3
                    b, h = units[j]
                    dst = attnT[h // 2][(h % 2) * HD:(h % 2 + 1) * HD,
                                        b * NQ:(b + 1) * NQ]
                    nc.vector.tensor_mul(dst, us[j][0:HD, :], bcs[j][:])
                    del us[j], bcs[j]
                for f in fillers.get(i, ()):
                    if f[0] == "q":
                        q_block(f[1], f[2])
                    elif f[0] == "r":
                        rope_half(f[1], f[2], pool=True)
                    else:
                        e_block(f[1], f[2])
            for nk in range(4):
                e_block(1, nk)

    nc.compile()
    return nc


# ======================= host-side preparation =======================

def _angles(dim, end, w, step=1.0, bias=0.0, theta=10000.0):
    flat = np.arange(end, dtype=np.float32)
    xp = (bias + (flat % w) * step).astype(np.float32)
    yp = (bias + (flat // w) * step).astype(np.float32)
    freqs = (1.0 / theta ** (np.arange(0, dim, 4, dtype=np.float32)[: dim // 4]
                             / dim)).astype(np.float32)
    xf = np.outer(xp, freqs)
    yf = np.outer(yp, freqs)
    return np.stack([xf, yf], axis=-1).reshape(end, -1).astype(np.float32)


def _host_prep(x, Wq, Wkv, sr_w, sr_b, ln_g, ln_b, proj_w, proj_b):
    f = np.float32
    x = np.asarray(x, f)
    Wq = np.asarray(Wq, f)
    Wkv = np.asarray(Wkv, f)
    sr_w = np.asarray(sr_w, f)
    proj_w = np.asarray(proj_w, f)
    sr_b = np.asarray(sr_b, f)
    ln_g = np.asarray(ln_g, f)
    ln_b = np.asarray(ln_b, f)
    proj_b = np.asarray(proj_b, f)

    # head-dim permutation: pair-first -> rows 0..383, pair-second -> 384..767
    hh = np.arange(HEADS)[:, None] * HD
    jj = np.arange(HD // 2)[None, :] * 2
    perm = np.concatenate([(hh + jj).ravel(), (hh + jj + 1).ravel()])

    Wk = Wkv[:C]
    Wv = Wkv[C:]
    wqT = np.ascontiguousarray(Wq[perm, :].T.astype(BF))
    wkT = np.ascontiguousarray((Wk * ln_g[None, :])[perm, :].T.astype(BF))
    wvT = np.ascontiguousarray((Wv * ln_g[None, :]).T.astype(BF))
    kbias = (Wk @ ln_b)[perm].astype(f)
    vbias = (Wv @ ln_b).astype(f)
    pbias_host = (proj_b + proj_w @ vbias).astype(f)
    srwT = np.ascontiguousarray(sr_w.reshape(C, KC).T.astype(BF))
    projT = np.ascontiguousarray(proj_w.T.astype(BF))

    # im2col of the first 6 image rows, both batches: [3072, 192] pad 256
    strip = x[:, :6 * W, :].reshape(B, 3, 2, 32, 2, C)   # b, i, di, j, dj, c
    xi2c = np.zeros((KC, 256), f)
    xi2c[:, :M2] = strip.transpose(5, 2, 4, 0, 1, 3).reshape(KC, M2)
    xi2c = xi2c.astype(BF)

    # RoPE tables
    ang_q = _angles(HD, N, W)
    ang_k = _angles(HD, N // (SR * SR), W, step=SR, bias=1.0 - 1.0 / SR)
    rowj = np.arange(128) % 32
    cq_full = np.cos(ang_q)[:, rowj].T                   # [128, 4096]
    sq_full = np.sin(ang_q)[:, rowj].T
    ckk = np.cos(ang_k)[:M, rowj].T                      # [128, 96]
    skk = np.sin(ang_k)[:M, rowj].T
    ck2 = np.concatenate([ckk, ckk], 1)
    sk2 = np.concatenate([skk, skk], 1)

    # visibility
    n_all = np.arange(N)
    xpos = n_all // (SR * H)
    ox = n_all // H
    oy = n_all % H
    ypos = (ox + oy * H) // (SR * H)
    vis = xpos * SR + ypos + 1                            # [4096], 1..94

    # mask matrices: A[j, m] = NEG*[m >= j]; B[j, n] = [vis(n) == j]
    A_m = np.zeros((128, M), f)
    jm = np.arange(M)
    A_m[:M, :] = np.where(jm[None, :] >= jm[:, None], NEG, 0.0)

    blob32 = np.zeros((128, 12), f)
    blob32[:, 0:6] = sr_b.reshape(NCH, 128).T
    blob32[:, 6:12] = kbias.reshape(NCH, 128).T

    shared = dict(xi2c=xi2c, wqT=wqT, wkT=wkT, wvT=wvT, srwT=srwT,
                  projT=projT, blob32=blob32)

    in_maps = []
    for core in range(N_CORES):
        ns = slice(core * NQ, (core + 1) * NQ)
        xs = x[:, ns, :]
        xTc = np.ascontiguousarray(
            xs.transpose(2, 0, 1).reshape(C, NQT).astype(BF))
        visc = vis[ns]
        B_m = np.zeros((128, NQ), f)
        B_m[:M, :] = (visc[None, :] == jm[:, None]).astype(f)
        blob = np.zeros((128, BLOB16), f)
        blob[:, O_CQ:O_CQ + NQ] = cq_full[:, ns]
        blob[:, O_CQ + NQ:O_CQ + NQT] = cq_full[:, ns]
        blob[:, O_SQ:O_SQ + NQ] = sq_full[:, ns]
        blob[:, O_SQ + NQ:O_SQ + NQT] = sq_full[:, ns]
        blob[:, O_CK:O_CK + M2] = ck2
        blob[:, O_SK:O_SK + M2] = sk2
        blob[:, O_A:O_A + M] = A_m
        blob[:, O_B:O_B + NQ] = B_m
        in_maps.append(dict(shared, xT=xTc, blob16=blob.astype(BF)))
    return in_maps, pbias_host


_NC_CACHE = {}


def _get_program():
    if "nc" not in _NC_CACHE:
        _NC_CACHE["nc"] = build_program()
    return _NC_CACHE["nc"]


def kernel(x, Wq, Wkv, sr_w, sr_b, ln_g, ln_b, proj_w, proj_b, H=None, W=None,
           _trace=False):
    nc = _get_program()
    in_maps, pbias_host = _host_prep(x, Wq, Wkv, sr_w, sr_b, ln_g, ln_b,
                                     proj_w, proj_b)
    res = run_bass_kernel_spmd(nc, in_maps, list(range(N_CORES)),
                               trace=_trace)
    kernel.last_result = res
    out = np.empty((B, N, C), np.float32)
    for core in range(N_CORES):
        y = np.asarray(res.results[core]["y"]).astype(np.float32)
        out[:, core * NQ:(core + 1) * NQ, :] = y.reshape(B, NQ, C)
    out += pbias_host[None, None, :]
    return out
